# revision 1
# baseline (speedup 1.0000x reference)
"""Trainium2 Bass kernel for AspectNeighborAttention (gnn_message_passing).

Pure data-parallel over batch: 32 batches -> 8 NeuronCores x 4 batches.
All weights replicated, host-converted to bf16 and host-PRE-TRANSPOSED into
the chunk-major [128, KC, *] lhsT/rhs layouts the TensorEngine wants, so the
device does plain contiguous DMAs only. dep is host-bf16 (halves HBM traffic).

Key structure (derived over several profiled iterations; 290us -> ~166us):
  * W-folding kills the nbr intermediate entirely:
      temp = attn @ (zs @ G0^T) + D @ G1 + zs @ WhZ^T - bertS
    with G0 = WhN @ WfZ and G1 = (WhN @ WfE)^T precomputed on host, and
    -bertS folded into the same PSUM accumulation via bertsT x (-I) chunk
    matmuls, so the blend is a single DVE scalar_tensor_tensor reading
    PSUM: out = upd * psum + bertS (upd = host-folded span & any-neighbor
    mask, broadcast to a PSUM column by a rank-1 PE matmul).
  * All but 2 small temp matmuls pre-accumulate before the D reduction
    finishes, so the post-softmax serial tail is ~2us.
  * wa_e is folded into the host f32->bf16 cast of dep (depW = dep*wa_e)
    and 1/wa_e into G1, so s_e = reduce_e(depW) is a single DVE
    tensor_reduce (axis=X is its hard 1x floor) with NO multiply, and
    D' @ G1' is exactly D @ G1 (the diagonal rescale cancels; zeros in
    wa_e are guarded). Pool gets NO multiplies: measured Pool
    TENSOR_TENSOR MULTIPLY carries a ~5-8us fixed overhead regardless of
    size (ADDs are fine at ~1-2 ns/elem).
  * D = reduce_j(attn * dep) in natural [i,j,e] layout: DVE mult in two
    j-halves (so the Pool tree can start after the first half), then a
    pairwise j-tree ping-ponged between two scratch tiles, split DVE
    rows [64:128) / Pool rows [0:64).
  * lrelu on DVE as max(0.01x, x); ACT keeps only Exp (no table thrash)
    plus PSUM->SBUF copies; softmax masking via additive C-shift, and
    attn off-neighbor entries are exact 0 through exp underflow (no mask
    multiply needed).
  * Engine-queue ordering matters (in-order queues): emission order is
    queue order; deferring ops that wait on long cross-engine chains
    avoids head-of-line blocking, but holding the D-tree ping-pong tiles
    across a batch boundary (fully-serial Pool tree + deferred blend)
    stalls the next batch's D-mult on the tile ring - keep the tree split
    and the blend inline.
The remaining wall is the Vector engine (~70% busy, <6us idle) plus a
hardware power throttle (~50% util cap for ~2/3 of the kernel, visible in
the profile summary; it also makes single-run A/B deltas under ~25us
unresolvable); se-reduce and the D-mult are at their measured floors.

The roll(z,-1)/roll(out,+1) pair is handled purely with shifted-row DMAs
(bert is sent pre-rolled as `berts`).
"""

import sys

for _p in ("/opt/trn_rl_repo",):
    if _p not in sys.path:
        sys.path.insert(0, _p)

import os
import numpy as np
import ml_dtypes

import concourse.bass as bass
import concourse.bacc as bacc_mod
import concourse.mybir as mybir
import concourse.tile as tile
from concourse.masks import make_identity

B, L, H, E = 32, 128, 768, 64
NCORES = 8
PB = B // NCORES  # batches per core
KC = H // 128     # 6 k-chunks
F32 = mybir.dt.float32
BF16 = mybir.dt.bfloat16
AF = mybir.ActivationFunctionType
OP = mybir.AluOpType
AX = mybir.AxisListType
MASK_SHIFT = 10000.0  # additive mask offset (see score masking)

_CACHED = {}

CFG = dict(
    dep_bufs=int(os.environ.get("K_DEP_BUFS", 3)),
    ttmp_bufs=int(os.environ.get("K_TTMP_BUFS", 3)),
    spool_bufs=int(os.environ.get("K_SPOOL_BUFS", 3)),
    opool_bufs=int(os.environ.get("K_OPOOL_BUFS", 3)),
    ptr_bufs=int(os.environ.get("K_PTR_BUFS", 3)),
    pbig_bufs=int(os.environ.get("K_PBIG_BUFS", 2)),
    jpd=int(os.environ.get("K_JPD", 20)),  # D-mult j-split: [0,jpd) Pool
    jp1=int(os.environ.get("K_JP1", 0)),  # s_e-mult j-split: [0,jp1) Pool
)


def _build(debug=False):
    nc = bacc_mod.Bacc("TRN2", target_bir_lowering=False, debug=False,
                       num_devices=NCORES)

    bert = nc.dram_tensor("berts", [PB, L, H], F32, kind="ExternalInput")
    bertsT = nc.dram_tensor("bertsT", [PB, 128, KC, 128], BF16,
                            kind="ExternalInput")
    dep = nc.dram_tensor("dep", [PB, L, L, E], BF16, kind="ExternalInput")
    adjf = nc.dram_tensor("adjf", [PB, L, L], BF16, kind="ExternalInput")
    vrow = nc.dram_tensor("vrow", [1, PB, 128], BF16, kind="ExternalInput")
    wzT_d = nc.dram_tensor("wzT", [128, KC, H], BF16, kind="ExternalInput")
    g0T_d = nc.dram_tensor("g0T", [128, KC, H], BF16, kind="ExternalInput")
    whzT_d = nc.dram_tensor("whzT", [128, KC, H], BF16, kind="ExternalInput")
    g1_d = nc.dram_tensor("g1", [E, H], BF16, kind="ExternalInput")
    w2T_d = nc.dram_tensor("w2T", [128, KC, 2], BF16, kind="ExternalInput")
    bzt = nc.dram_tensor("bzt", [1, H], BF16, kind="ExternalInput")
    bat = nc.dram_tensor("bat", [1, 1], F32, kind="ExternalInput")
    out = nc.dram_tensor("out", [PB, L, H], F32, kind="ExternalOutput")

    dbg = {}
    if debug:
        for nm, shape, dt in [
            ("d_zsT", [128, KC, 128], BF16), ("d_si", [1, 128], F32),
            ("d_sjb", [1, 128], F32), ("d_se", [128, L], BF16),
            ("d_masked", [128, L], F32), ("d_attn", [128, L], BF16),
            ("d_dvec", [128, E], BF16), ("d_ab", [128, H], BF16),
            ("d_nbrT", [128, KC, 128], BF16), ("d_tempb", [128, H], F32),
            ("d_upd", [128, 1], F32), ("d_scb", [128, 128], F32),
        ]:
            dbg[nm] = nc.dram_tensor(nm, shape, dt, kind="ExternalOutput")
    with tile.TileContext(nc) as tc:
        with nc.allow_low_precision("bf16 softmax/D path, 2e-2 rel-err gate"):
            _body(tc, nc, bert, bertsT, dep, adjf, vrow, wzT_d, g0T_d,
                  whzT_d, g1_d, w2T_d, bzt, bat, out, dbg)
    nc.compile()
    return nc


def _body(tc, nc, bert, bertsT, dep, adjf, vrow, wzT_d, g0T_d,
          whzT_d, g1_d, w2T_d, bzt, bat, out, dbg=None):
    def dump(name, ap):
        if dbg and name in dbg:
            nc.sync.dma_start(dbg[name][...], ap)
    import contextlib
    cfg = CFG
    JPD = cfg["jpd"]
    JP1 = cfg["jp1"]
    ctx = contextlib.ExitStack()
    with ctx:
        wpool = ctx.enter_context(tc.tile_pool(name="weights", bufs=1))
        dpool = ctx.enter_context(
            tc.tile_pool(name="dep", bufs=cfg["dep_bufs"]))
        tpool = ctx.enter_context(
            tc.tile_pool(name="ttmp", bufs=cfg["ttmp_bufs"]))
        spool = ctx.enter_context(
            tc.tile_pool(name="small", bufs=cfg["spool_bufs"]))
        opool = ctx.enter_context(
            tc.tile_pool(name="outp", bufs=cfg["opool_bufs"]))
        p_tr = ctx.enter_context(
            tc.tile_pool(name="p_tr", bufs=cfg["ptr_bufs"], space="PSUM"))
        p_big = ctx.enter_context(
            tc.tile_pool(name="p_big", bufs=cfg["pbig_bufs"], space="PSUM"))

        # ---------------- one-time setup (plain DMAs only) ----------------
        wzT = wpool.tile([128, KC, H], BF16, tag="wzT")
        nc.sync.dma_start(wzT[:], wzT_d[...])
        g0T = wpool.tile([128, KC, H], BF16, tag="g0T")
        nc.sync.dma_start(g0T[:], g0T_d[...])
        whzT = wpool.tile([128, KC, H], BF16, tag="whzT")
        nc.sync.dma_start(whzT[:], whzT_d[...])
        g1 = wpool.tile([E, H], BF16, tag="g1")
        nc.sync.dma_start(g1[:], g1_d[...])
        w2T = wpool.tile([128, KC, 2], BF16, tag="w2T")
        nc.sync.dma_start(w2T[:], w2T_d[...])
        bzr = wpool.tile([1, H], BF16, tag="bzr")
        nc.sync.dma_start(bzr[:], bzt[:, :])
        bar = wpool.tile([1, 1], F32, tag="bar")
        nc.sync.dma_start(bar[:], bat[:, :])
        vrow4 = wpool.tile([1, PB, 128], BF16, tag="vrow4")
        nc.sync.dma_start(vrow4[:], vrow[:, :, :])

        ones_f = wpool.tile([1, 128], F32, tag="ones_f")
        nc.gpsimd.memset(ones_f[:], 1.0)
        ones_b = wpool.tile([1, 128], BF16, tag="ones_b")
        nc.gpsimd.memset(ones_b[:], 1.0)
        id_bf = wpool.tile([128, 128], BF16, tag="id_bf")
        make_identity(nc, id_bf[:])
        id_negb = wpool.tile([128, 128], BF16, tag="id_negb")
        nc.vector.tensor_scalar(id_negb[:], id_bf[:], -1.0, None, op0=OP.mult)

        # -------- per-batch pipeline, software-pipelined --------
        # The blend/store of batch b-1 is emitted mid-iteration-b so it
        # never head-of-line-blocks the next batch's front-end work on the
        # in-order engine queues.
        def emit_front(b):
            st = {}
            # bertS: rows shifted by one token (z roll); f32 exact for blend
            bertS = spool.tile([128, H], F32, tag="bertS")
            nc.sync.dma_start(bertS[:], bert[b, :, :])
            bertST = spool.tile([128, KC, 128], BF16, tag="bertST")
            nc.sync.dma_start(bertST[:], bertsT[b, :, :, :])
            st["bertST"] = bertST
            dept = dpool.tile([128, L, E], BF16, tag="dept")
            nc.sync.dma_start(dept[:], dep[b, :, :, :])
            adjt = spool.tile([128, L], BF16, tag="adjt")
            nc.sync.dma_start(adjt[:], adjf[b, :, :])
            st.update(bertS=bertS, dept=dept, adjt=adjt)

            # ---- zs^T = Wz @ bertS^T + bz ----
            p_z = p_big.tile([128, H], F32, tag="p_big")
            for hc in range(KC):
                ns = slice(hc * 128, (hc + 1) * 128)
                for kc in range(KC):
                    nc.tensor.matmul(p_z[:, ns], wzT[:, kc, ns],
                                     bertST[:, kc, :],
                                     start=(kc == 0), stop=False)
                nc.tensor.matmul(p_z[:, ns], bzr[0:1, ns], ones_b[:],
                                 start=False, stop=True)
            zsT = spool.tile([128, KC, 128], BF16, tag="zsT")
            nc.scalar.copy(zsT[:], p_z[:])
            if b == 0:
                dump("d_zsT", zsT[:])

            # ---- s_i col, (s_j + ba) row-bcast score base ----
            p_s3 = p_tr.tile([128, 512], F32, tag="p_tr")
            for kc in range(KC):
                nc.tensor.matmul(p_s3[0:1, 0:128], w2T[:, kc, 0:1],
                                 zsT[:, kc, :],
                                 start=(kc == 0), stop=(kc == KC - 1))
            for kc in range(KC):
                nc.tensor.matmul(p_s3[0:1, 128:256], w2T[:, kc, 1:2],
                                 zsT[:, kc, :],
                                 start=(kc == 0), stop=(kc == KC - 1))
            si_row = spool.tile([1, 128], F32, tag="si_row")
            nc.scalar.copy(si_row[:], p_s3[0:1, 0:128])
            sjb = spool.tile([1, 128], F32, tag="sjb")
            nc.vector.tensor_scalar(sjb[:], p_s3[0:1, 128:256], bar[0:1, 0:1],
                                    None, op0=OP.add)
            nc.tensor.matmul(p_s3[:, 384:385], si_row[:], ones_f[0:1, 0:1],
                             start=True, stop=True)
            nc.tensor.matmul(p_s3[:, 256:384], ones_f[:], sjb[:],
                             start=True, stop=True)
            if b == 0:
                dump("d_si", si_row[:])
                dump("d_sjb", sjb[:])

            # ---- s_e = reduce_e(depW): wa_e is host-folded into dep,
            # and 1/wa_e into G1, so no multiply is needed here at all ----
            se = spool.tile([128, L], BF16, tag="se")
            nc.vector.tensor_reduce(se[:], dept[:], axis=AX.X, op=OP.add)
            if b == 0:
                dump("d_se", se[:])

            # ---- score = lrelu(se + si + sj + ba); masked; softmax ----
            sadd = spool.tile([128, L], F32, tag="sadd")
            nc.vector.scalar_tensor_tensor(
                sadd[:], se[:], p_s3[:, 384:385], p_s3[:, 256:384],
                op0=OP.add, op1=OP.add)
            score = spool.tile([128, L], F32, tag="score")
            nc.vector.scalar_tensor_tensor(
                score[:], sadd[:], 0.01, sadd[:], op0=OP.mult, op1=OP.max)
            masked = spool.tile([128, L], F32, tag="masked")
            nc.vector.scalar_tensor_tensor(
                masked[:], score[:], MASK_SHIFT, adjt[:],
                op0=OP.add, op1=OP.mult)
            if b == 0:
                dump("d_masked", masked[:])
            mxn = spool.tile([128, 1], F32, tag="mxn")
            nc.vector.tensor_reduce(mxn[:], masked[:], axis=AX.X, op=OP.max,
                                    negate=True)
            ex = spool.tile([128, L], F32, tag="ex")
            sumex = spool.tile([128, 1], F32, tag="sumex")
            nc.scalar.activation(ex[:], masked[:], AF.Exp, bias=mxn[:],
                                 scale=1.0, accum_out=sumex[:])
            rec = spool.tile([128, 1], F32, tag="rec")
            nc.vector.reciprocal(rec[:], sumex[:])
            attnb = spool.tile([128, L], BF16, tag="attnb")
            nc.vector.tensor_scalar(attnb[:], ex[:], rec[0:128, 0:1], None,
                                    op0=OP.mult)
            if b == 0:
                dump("d_attn", attnb[:])

            # attn^T transpose early (needs only attnb)
            p_ad = p_tr.tile([128, 256], BF16, tag="p_tr")
            nc.tensor.transpose(p_ad[:, 0:128], attnb[:], id_bf[:])
            attnT = spool.tile([128, 128], BF16, tag="attnT")
            nc.scalar.copy(attnT[:], p_ad[:, 0:128])
            st.update(zsT=zsT, attnb=attnb, p_ad=p_ad, attnT=attnT)
            return st

        def emit_back(b, st):
            zsT, dept, attnb = st["zsT"], st["dept"], st["attnb"]
            # ---- A2 = zs @ G0^T  (G0 = WhN @ WfZ host-folded) ----
            p_a = p_big.tile([128, H], F32, tag="p_big")
            for ns in (slice(0, 512), slice(512, H)):
                for kc in range(KC):
                    nc.tensor.matmul(p_a[:, ns], zsT[:, kc, :],
                                     g0T[:, kc, ns],
                                     start=(kc == 0), stop=(kc == KC - 1))
            a2b = spool.tile([128, H], BF16, tag="ab")
            nc.scalar.copy(a2b[:], p_a[:])

            # ---- temp partial: -bertS + zs@WhZ^T + attn@A2 (early) ----
            # temp(b) accumulates (temp - bertS) so the blend is one stt.
            p_t = p_big.tile([128, H], F32, tag="p_big")
            bertST_t = st["bertST"]
            for ns in (slice(0, 512), slice(512, H)):
                for kc in range(KC):
                    nc.tensor.matmul(p_t[:, ns], zsT[:, kc, :],
                                     whzT[:, kc, ns],
                                     start=(kc == 0), stop=False)
                nc.tensor.matmul(p_t[:, ns], st["attnT"][:], a2b[:, ns],
                                 start=False, stop=False)
            for hc in range(KC):
                ns = slice(hc * 128, (hc + 1) * 128)
                nc.tensor.matmul(p_t[:, ns], bertST_t[:, hc, :], id_negb[:],
                                 start=False, stop=False)

            # ---- D = reduce_j(attn * dep); Pool share densified via ACT ----
            tmp2 = tpool.tile([128, L, E], BF16, tag="ttmp")
            nc.vector.tensor_tensor(
                tmp2[:, 0:64, :], dept[:, 0:64, :],
                attnb[:, 0:64].unsqueeze(2).broadcast_to([128, 64, E]),
                op=OP.mult)
            nc.vector.tensor_tensor(
                tmp2[:, 64:L, :], dept[:, 64:L, :],
                attnb[:, 64:L].unsqueeze(2).broadcast_to([128, 64, E]),
                op=OP.mult)
            # asymmetric pairwise tree: DVE folds j[0:96), Pool j[96:128)
            tmp3 = tpool.tile([128, L, E], BF16, tag="ttmp")
            t2, t3 = tmp2, tmp3
            nc.vector.tensor_tensor(t3[:, 64:96, :], t2[:, 64:96, :],
                                    t2[:, 96:128, :], op=OP.add)
            nc.vector.tensor_tensor(t2[:, 64:80, :], t3[:, 64:80, :],
                                    t3[:, 80:96, :], op=OP.add)
            nc.vector.tensor_tensor(t3[:, 64:72, :], t2[:, 64:72, :],
                                    t2[:, 72:80, :], op=OP.add)
            nc.vector.tensor_tensor(t2[:, 64:68, :], t3[:, 64:68, :],
                                    t3[:, 68:72, :], op=OP.add)
            nc.vector.tensor_tensor(t3[:, 64:66, :], t2[:, 64:66, :],
                                    t2[:, 66:68, :], op=OP.add)
            nc.vector.tensor_tensor(t2[:, 64:65, :], t3[:, 64:65, :],
                                    t3[:, 65:66, :], op=OP.add)
            nc.gpsimd.tensor_tensor(t3[:, 0:32, :], t2[:, 0:32, :],
                                    t2[:, 32:64, :], op=OP.add)
            nc.gpsimd.tensor_tensor(t2[:, 0:16, :], t3[:, 0:16, :],
                                    t3[:, 16:32, :], op=OP.add)
            nc.gpsimd.tensor_tensor(t3[:, 0:8, :], t2[:, 0:8, :],
                                    t2[:, 8:16, :], op=OP.add)
            nc.gpsimd.tensor_tensor(t2[:, 0:4, :], t3[:, 0:4, :],
                                    t3[:, 4:8, :], op=OP.add)
            nc.gpsimd.tensor_tensor(t3[:, 0:2, :], t2[:, 0:2, :],
                                    t2[:, 2:4, :], op=OP.add)
            nc.gpsimd.tensor_tensor(t2[:, 0:1, :], t3[:, 0:1, :],
                                    t3[:, 1:2, :], op=OP.add)
            dvb = spool.tile([128, E], BF16, tag="dvb")
            nc.vector.tensor_tensor(dvb[:], t2[:, 0, :], t2[:, 64, :],
                                    op=OP.add)
            if b == 0:
                dump("d_dvec", dvb[:])

            # D^T via PE transpose
            p_ad = st["p_ad"]
            nc.tensor.transpose(p_ad[0:E, 128:256], dvb[:], id_bf[:])
            dT = spool.tile([E, 128], BF16, tag="dT")
            nc.scalar.copy(dT[:], p_ad[0:E, 128:256])

            # ---- temp final: += D @ G1 (G1 = (WhN @ WfE)^T host-folded) ----
            for ns in (slice(0, 512), slice(512, H)):
                nc.tensor.matmul(p_t[:, ns], dT[:], g1[:, ns],
                                 start=False, stop=True)
            st["p_t"] = p_t

            # ---- upd mask column (host-folded span & any-neighbor) ----
            p_v = p_tr.tile([128, 512], F32, tag="p_tr")
            nc.tensor.matmul(p_v[:, 0:1], vrow4[0:1, b, :], ones_b[0:1, 0:1],
                             start=True, stop=True)
            # ---- blend: out = upd*(temp - bertS) + bertS; rolled store ----
            outt = opool.tile([128, H], F32, tag="outt")
            nc.vector.scalar_tensor_tensor(
                outt[:], p_t[:], p_v[:, 0:1], st["bertS"][:],
                op0=OP.mult, op1=OP.add)
            nc.sync.dma_start(out[b, 1:128, :], outt[0:127, :])
            nc.sync.dma_start(out[b, 0:1, :], outt[127:128, :])

        for b in range(PB):
            st = emit_front(b)
            emit_back(b, st)


def _get_nc():
    if "nc" not in _CACHED:
        _CACHED["nc"] = _build()
    return _CACHED["nc"]


def _chunkT(w):
    """W [rows, K] -> W^T chunk-major [128, K//128, rows] (lhsT layout)."""
    rows, k = w.shape
    return np.ascontiguousarray(
        w.T.reshape(k // 128, 128, rows).transpose(1, 0, 2))


def _prep_in_maps(bert_hidden_states, dep_type_adj, deprel_adj,
                  asp_start, asp_end, Wz, bz, wa, ba, Wf, Wh):
    bf = ml_dtypes.bfloat16
    bert = np.ascontiguousarray(np.asarray(bert_hidden_states, np.float32))
    wa_f0 = np.asarray(wa, np.float32)
    wae_f = wa_f0[2 * H:]
    wae_safe = np.where(wae_f == 0.0, 1.0, wae_f)
    dep = (np.asarray(dep_type_adj, np.float32)
           * wae_f[None, None, None, :]).astype(bf)
    adjf = np.ascontiguousarray(np.asarray(deprel_adj).astype(ml_dtypes.bfloat16))
    # bertS^T chunk-major per batch: rows shifted by one (the z-roll)
    bs = np.ascontiguousarray(np.roll(bert, -1, axis=1))
    bertsT = np.ascontiguousarray(
        bs.transpose(0, 2, 1).reshape(B, KC, 128, L).transpose(0, 2, 1, 3)
    ).astype(bf)
    pos = np.arange(L, dtype=np.float32)
    s_ = np.asarray(asp_start).astype(np.float32)[:, None]
    e_ = np.asarray(asp_end).astype(np.float32)[:, None]
    vrow_full = (((pos[None, :] >= s_) & (pos[None, :] <= e_))
                 & (np.asarray(deprel_adj) > 0).any(-1)).astype(ml_dtypes.bfloat16)

    Wz = np.asarray(Wz, np.float32)
    Wf = np.asarray(Wf, np.float32)
    Wh = np.asarray(Wh, np.float32)
    wa_f = wa_f0
    wzT = _chunkT(Wz).astype(bf)
    g0T = _chunkT(Wh[:, :H] @ Wf[:, :H]).astype(bf)
    whzT = _chunkT(Wh[:, H:]).astype(bf)
    g1 = np.ascontiguousarray(
        (Wh[:, :H] @ Wf[:, H:]).T / wae_safe[:, None]).astype(bf)
    w2T = _chunkT(wa_f[:2 * H].reshape(2, H)).astype(bf)
    bzb = np.asarray(bz, np.float32)[None, :].astype(bf)
    bab = np.asarray(ba, np.float32).reshape(1, 1)

    in_maps = []
    for c in range(NCORES):
        s = slice(c * PB, (c + 1) * PB)
        in_maps.append(dict(
            berts=bs[s], bertsT=np.ascontiguousarray(bertsT[s]),
            dep=dep[s], adjf=adjf[s],
            vrow=np.ascontiguousarray(vrow_full[s][None, :, :]),
            wzT=wzT, g0T=g0T, whzT=whzT, g1=g1, w2T=w2T,
            bzt=bzb, bat=bab,
        ))
    return in_maps


def kernel(bert_hidden_states, dep_type_adj, deprel_adj, asp_start, asp_end,
           Wz, bz, wa, ba, Wf, Wh):
    from concourse.bass_utils import run_bass_kernel_spmd

    in_maps = _prep_in_maps(bert_hidden_states, dep_type_adj, deprel_adj,
                            asp_start, asp_end, Wz, bz, wa, ba, Wf, Wh)
    nc = _get_nc()
    res = run_bass_kernel_spmd(nc, in_maps, core_ids=list(range(NCORES)),
                               trace=bool(_CACHED.get("trace")),
                               tmpdir=_CACHED.get("trace_tmpdir"))
    _CACHED["last_results"] = res
    outs = [res.results[c]["out"] for c in range(NCORES)]
    return np.concatenate(outs, axis=0).astype(np.float32)



# revision 11
# speedup vs baseline: 1.0808x; 1.0808x over previous
"""Trainium2 Bass kernel for AspectNeighborAttention (gnn_message_passing).

Pure data-parallel over batch: 32 batches -> 8 NeuronCores x 4 batches.
All weights replicated, host-converted to bf16 and host-PRE-TRANSPOSED into
the chunk-major [128, KC, *] lhsT/rhs layouts the TensorEngine wants.

v2 redesign (from v1 at ~167us measured):
  * The zs GEMM is gone entirely: every consumer of zs is host-folded onto
    bertS directly (A2 = bertS @ (G0 Wz)^T, temp1 = bertS @ (WhZ Wz - I)^T
    which also folds the -bertS blend term, s_i/s_j = bertS @ (Wz^T wa_*)
    with the bz constants folded into ba / a bias row).  PE instruction
    count halves and the serial zsT dependency disappears.
  * dep is host-transposed to [i, e, j] (wa_e pre-folded, bf16; 1/wa_e
    folded into G1 rows so D' @ G1' == D @ G1 exactly):
      - s_e = sum_e dep': binary TT-add tree over the MIDDLE e axis.
        TensorReduce has NO DVE fast modes (1.04 ns/elem always) but
        TensorTensor has 2x_1p (0.52 ns/elem when every operand is 2-byte
        with innermost stride 1), so a 6-level tree (8064 elems) beats the
        single reduce (8192 elems) 2:1.
      - D-mult tmp = dep' * attn[i,j]-broadcast: the broadcast is over the
        middle axis so the innermost stride stays 1 -> 2x mode (v1's
        innermost-stride-0 broadcast forced 1x).
      - D-reduce over the innermost j axis: 7-level TT tree at 2x.
    DVE work per batch drops ~22us -> ~12us (cost-model validated).
  * Masking is additive and PE-folded: host sends madd^T (0 / -1e4), a
    maddT x Identity matmul accumulates it into the same PSUM tile as the
    s_j row broadcast, so score = lrelu(se + si_col + sjmadd) needs only
    2 DVE STTs; masked entries hit exp underflow -> exact 0 attn.  The
    row-max pass is dropped (scores are bounded ~+-8, exp is safe); sumex
    gets +1e-30 so all-masked rows yield attn=0 instead of NaN.
  * attn = ex * rec as a bf16 tensor_scalar (4x_2p mode, ~33ns).
  * GpSimd carries a balanced slice of each tree level + the e-tail of the
    D path; emission is software-pipelined (back(b-1) emitted after
    front(b)) so dT/G1/blend of batch b-1 never head-of-line-block batch
    b's PE/DVE front-end work.  PSUM: 3x p_big(2 banks) + 2x p_x(1) = 8.
"""

import sys

for _p in ("/opt/trn_rl_repo",):
    if _p not in sys.path:
        sys.path.insert(0, _p)

import os
import numpy as np
import ml_dtypes

import concourse.bass as bass
import concourse.bacc as bacc_mod
import concourse.mybir as mybir
import concourse.tile as tile
from concourse.masks import make_identity

B, L, H, E = 32, 128, 768, 64
NCORES = 8
PB = B // NCORES  # batches per core
KC = H // 128     # 6 k-chunks
F32 = mybir.dt.float32
BF16 = mybir.dt.bfloat16
AF = mybir.ActivationFunctionType
OP = mybir.AluOpType
AX = mybir.AxisListType
MASK_NEG = -10000.0

_CACHED = {}

CFG = dict(
    dep_bufs=int(os.environ.get("K_DEP_BUFS", 3)),
    tmpd_bufs=int(os.environ.get("K_TMPD_BUFS", 2)),
    spool_bufs=int(os.environ.get("K_SPOOL_BUFS", 3)),
    opool_bufs=int(os.environ.get("K_OPOOL_BUFS", 2)),
    px_bufs=int(os.environ.get("K_PX_BUFS", 2)),
    pbig_bufs=int(os.environ.get("K_PBIG_BUFS", 2)),
    esplit=int(os.environ.get("K_ESPLIT", 56)),   # D path: e[0:es) DVE, rest GpSimd
    jsplit=int(os.environ.get("K_JSPLIT", 96)),   # se L1/L2: j[0:js) DVE, rest GpSimd
)


def _build(debug=False):
    nc = bacc_mod.Bacc("TRN2", target_bir_lowering=False, debug=False,
                       num_devices=NCORES)

    bert = nc.dram_tensor("berts", [PB, L, H], F32, kind="ExternalInput")
    bertsT = nc.dram_tensor("bertsT", [PB, 128, KC, 128], BF16,
                            kind="ExternalInput")
    dept_d = nc.dram_tensor("dept", [PB, 128, E, 128], BF16,
                            kind="ExternalInput")
    maddT_d = nc.dram_tensor("maddT", [PB, 128, 128], BF16,
                             kind="ExternalInput")
    vrow = nc.dram_tensor("vrow", [1, PB, 128], BF16, kind="ExternalInput")
    g0wT_d = nc.dram_tensor("g0wT", [128, KC, H], BF16, kind="ExternalInput")
    m1T_d = nc.dram_tensor("m1T", [128, KC, H], BF16, kind="ExternalInput")
    g1_d = nc.dram_tensor("g1", [E, H], BF16, kind="ExternalInput")
    u2T_d = nc.dram_tensor("u2T", [128, KC, 2], BF16, kind="ExternalInput")
    browt = nc.dram_tensor("browt", [1, H], BF16, kind="ExternalInput")
    bat = nc.dram_tensor("bat", [1, 1], F32, kind="ExternalInput")
    out = nc.dram_tensor("out", [PB, L, H], F32, kind="ExternalOutput")

    dbg = {}
    if debug:
        for nm, shape, dt in [
            ("d_si", [1, 128], F32), ("d_sjb", [1, 128], F32),
            ("d_se", [128, L], BF16), ("d_sadd", [128, L], F32),
            ("d_attn", [128, L], BF16), ("d_dvec", [128, E], BF16),
            ("d_a2b", [128, H], BF16), ("d_upd", [128, 1], F32),
        ]:
            dbg[nm] = nc.dram_tensor(nm, shape, dt, kind="ExternalOutput")
    with tile.TileContext(nc) as tc:
        with nc.allow_low_precision("bf16 softmax/D path, 2e-2 rel-err gate"):
            _body(tc, nc, bert, bertsT, dept_d, maddT_d, vrow, g0wT_d, m1T_d,
                  g1_d, u2T_d, browt, bat, out, dbg)
    nc.compile()
    return nc


def _body(tc, nc, bert, bertsT, dept_d, maddT_d, vrow, g0wT_d, m1T_d,
          g1_d, u2T_d, browt, bat, out, dbg=None):
    def dump(name, ap):
        if dbg and name in dbg:
            nc.sync.dma_start(dbg[name][...], ap)
    import contextlib
    cfg = CFG
    ES = cfg["esplit"]
    JS = cfg["jsplit"]
    ctx = contextlib.ExitStack()
    with ctx:
        wpool = ctx.enter_context(tc.tile_pool(name="weights", bufs=1))
        dpool = ctx.enter_context(
            tc.tile_pool(name="dep", bufs=cfg["dep_bufs"]))
        tpool = ctx.enter_context(
            tc.tile_pool(name="tmpd", bufs=cfg["tmpd_bufs"]))
        spool = ctx.enter_context(
            tc.tile_pool(name="small", bufs=cfg["spool_bufs"]))
        opool = ctx.enter_context(
            tc.tile_pool(name="outp", bufs=cfg["opool_bufs"]))
        # PSUM budget (8 banks): p_t [128,1024]f32 = 2 banks x2 bufs,
        # p_a [128,512]f32 = 1 bank x2, ptb [128,256]bf16 = 1 bank x2.
        p_apool = ctx.enter_context(
            tc.tile_pool(name="p_a", bufs=cfg["px_bufs"], space="PSUM"))
        p_tb = ctx.enter_context(
            tc.tile_pool(name="p_tb", bufs=cfg["px_bufs"], space="PSUM"))
        p_big = ctx.enter_context(
            tc.tile_pool(name="p_big", bufs=cfg["pbig_bufs"], space="PSUM"))

        # ---------------- one-time setup (plain DMAs only) ----------------
        g0wT = wpool.tile([128, KC, H], BF16, tag="g0wT")
        nc.sync.dma_start(g0wT[:], g0wT_d[...])
        m1T = wpool.tile([128, KC, H], BF16, tag="m1T")
        nc.sync.dma_start(m1T[:], m1T_d[...])
        g1 = wpool.tile([E, H], BF16, tag="g1")
        nc.sync.dma_start(g1[:], g1_d[...])
        u2T = wpool.tile([128, KC, 2], BF16, tag="u2T")
        nc.sync.dma_start(u2T[:], u2T_d[...])
        brow = wpool.tile([1, H], BF16, tag="brow")
        nc.sync.dma_start(brow[:], browt[:, :])
        bar = wpool.tile([1, 1], F32, tag="bar")
        nc.sync.dma_start(bar[:], bat[:, :])
        vrow4 = wpool.tile([1, PB, 128], BF16, tag="vrow4")
        nc.sync.dma_start(vrow4[:], vrow[:, :, :])

        ones_f = wpool.tile([1, 128], F32, tag="ones_f")
        nc.gpsimd.memset(ones_f[:], 1.0)
        ones_b = wpool.tile([1, 128], BF16, tag="ones_b")
        nc.gpsimd.memset(ones_b[:], 1.0)
        id_bf = wpool.tile([128, 128], BF16, tag="id_bf")
        make_identity(nc, id_bf[:])

        # -------- per-batch pipeline --------
        def emit_front(b):
            st = {}
            dept = dpool.tile([128, E, 128], BF16, tag="dept")
            nc.sync.dma_start(dept[:], dept_d[b, :, :, :])
            bertS = spool.tile([128, H], F32, tag="bertS")
            nc.sync.dma_start(bertS[:], bert[b, :, :])
            bertST = spool.tile([128, KC, 128], BF16, tag="bertST")
            nc.sync.dma_start(bertST[:], bertsT[b, :, :, :])
            maddT = spool.tile([128, 128], BF16, tag="maddT")
            nc.sync.dma_start(maddT[:], maddT_d[b, :, :])
            st.update(bertS=bertS, dept=dept)

            # ---- A2 = bertS @ (G0 Wz)^T (two chunks through a 1-bank ring) --
            a2b = spool.tile([128, H], BF16, tag="a2b")
            for ns in (slice(0, 512), slice(512, H)):
                p_a = p_apool.tile([128, 512], F32, tag="p_a")
                w = ns.stop - ns.start
                for kc in range(KC):
                    nc.tensor.matmul(p_a[:, 0:w], bertST[:, kc, :],
                                     g0wT[:, kc, ns],
                                     start=(kc == 0), stop=(kc == KC - 1))
                nc.scalar.copy(a2b[:, ns], p_a[:, 0:w])
            if b == 0:
                dump("d_a2b", a2b[:])

            # p_t [128,1024] = 2 banks: [0:768] temp accum; [768:896] s_i row
            # then (WAR) sj+madd bcast; [896:1024] s_j row then (WAR) si col
            # at 1023 and upd col at 1022.
            p_t = p_big.tile([128, 1024], F32, tag="p_big")

            # ---- s_i / s_j rows (m=1 each; DVE can't read partition 1) ----
            for kc in range(KC):
                nc.tensor.matmul(p_t[0:1, 768:896], u2T[:, kc, 0:1],
                                 bertST[:, kc, :],
                                 start=(kc == 0), stop=(kc == KC - 1))
            for kc in range(KC):
                nc.tensor.matmul(p_t[0:1, 896:1024], u2T[:, kc, 1:2],
                                 bertST[:, kc, :],
                                 start=(kc == 0), stop=(kc == KC - 1))
            si_row = spool.tile([1, 128], F32, tag="si_row")
            nc.scalar.copy(si_row[:], p_t[0:1, 768:896])
            sjb = spool.tile([1, 128], F32, tag="sjb")
            nc.vector.tensor_scalar(sjb[:], p_t[0:1, 896:1024], bar[0:1, 0:1],
                                    None, op0=OP.add)
            # sj row bcast + additive mask (WAR over s_i region), si col
            nc.tensor.matmul(p_t[:, 768:896], maddT[:], id_bf[:],
                             start=True, stop=False)
            nc.tensor.matmul(p_t[:, 768:896], ones_f[:], sjb[:],
                             start=False, stop=True)
            nc.tensor.matmul(p_t[:, 1023:1024], si_row[:], ones_f[0:1, 0:1],
                             start=True, stop=True)
            if b == 0:
                dump("d_si", si_row[:])
                dump("d_sjb", sjb[:])

            # ---- temp1 = bertS @ (WhZ Wz - I)^T + brow ----
            for ns in (slice(0, 512), slice(512, H)):
                for kc in range(KC):
                    nc.tensor.matmul(p_t[:, ns], bertST[:, kc, :],
                                     m1T[:, kc, ns],
                                     start=(kc == 0), stop=False)
                nc.tensor.matmul(p_t[:, ns], ones_b[:], brow[0:1, ns],
                                 start=False, stop=False)

            # ---- s_e: 6-level TT tree over middle e axis (2x mode) ----
            seA = tpool.tile([128, 32, 128], BF16, tag="seA")
            seB = tpool.tile([128, 16, 128], BF16, tag="seB")
            nc.vector.tensor_tensor(seA[:, :, 0:JS], dept[:, 0:32, 0:JS],
                                    dept[:, 32:64, 0:JS], op=OP.add)
            nc.gpsimd.tensor_tensor(seA[:, :, JS:128], dept[:, 0:32, JS:128],
                                    dept[:, 32:64, JS:128], op=OP.add)
            nc.vector.tensor_tensor(seB[:, :, 0:JS], seA[:, 0:16, 0:JS],
                                    seA[:, 16:32, 0:JS], op=OP.add)
            nc.gpsimd.tensor_tensor(seB[:, :, JS:128], seA[:, 0:16, JS:128],
                                    seA[:, 16:32, JS:128], op=OP.add)
            nc.vector.tensor_tensor(seA[:, 0:8, :], seB[:, 0:8, :],
                                    seB[:, 8:16, :], op=OP.add)
            nc.vector.tensor_tensor(seB[:, 0:4, :], seA[:, 0:4, :],
                                    seA[:, 4:8, :], op=OP.add)
            nc.vector.tensor_tensor(seA[:, 0:2, :], seB[:, 0:2, :],
                                    seB[:, 2:4, :], op=OP.add)
            sef = spool.tile([128, 128], BF16, tag="sef")
            nc.vector.tensor_tensor(sef[:], seA[:, 0, :], seA[:, 1, :],
                                    op=OP.add)
            if b == 0:
                dump("d_se", sef[:])

            # ---- score = lrelu(se + si + sj + madd); softmax (no rowmax) ----
            sadd = spool.tile([128, L], F32, tag="sadd")
            nc.vector.scalar_tensor_tensor(
                sadd[:], sef[:], p_t[:, 1023:1024], p_t[:, 768:896],
                op0=OP.add, op1=OP.add)
            score = spool.tile([128, L], F32, tag="score")
            nc.vector.scalar_tensor_tensor(
                score[:], sadd[:], 0.01, sadd[:], op0=OP.mult, op1=OP.max)
            if b == 0:
                dump("d_sadd", score[:])
            ex = spool.tile([128, L], BF16, tag="ex")
            sumex = spool.tile([128, 1], F32, tag="sumex")
            nc.scalar.activation(ex[:], score[:], AF.Exp, bias=0.0,
                                 scale=1.0, accum_out=sumex[:])
            sume = spool.tile([128, 1], F32, tag="sume")
            nc.vector.tensor_scalar(sume[:], sumex[:], 1e-30, None,
                                    op0=OP.add)
            rec = spool.tile([128, 1], F32, tag="rec")
            nc.vector.reciprocal(rec[:], sume[:])
            attnb = spool.tile([128, L], BF16, tag="attnb")
            nc.vector.tensor_scalar(attnb[:], ex[:], rec[0:128, 0:1], None,
                                    op0=OP.mult)
            if b == 0:
                dump("d_attn", attnb[:])

            # attn^T via PE (bf16 PSUM ring shared with the dT transpose)
            ptb = p_tb.tile([128, 256], BF16, tag="p_tb")
            nc.tensor.transpose(ptb[:, 0:128], attnb[:], id_bf[:])
            attnT = spool.tile([128, 128], BF16, tag="attnT")
            nc.scalar.copy(attnT[:], ptb[:, 0:128])

            # ---- D-mult: tmp[i,e,j] = dep'[i,e,j] * attn[i,j] (2x) ----
            tmpD = tpool.tile([128, E, 128], BF16, tag="tmpD")
            nc.vector.tensor_tensor(
                tmpD[:, 0:ES, :], dept[:, 0:ES, :],
                attnb[:].unsqueeze(1).broadcast_to([128, ES, 128]),
                op=OP.mult)
            nc.gpsimd.tensor_tensor(
                tmpD[:, ES:E, :], dept[:, ES:E, :],
                attnb[:].unsqueeze(1).broadcast_to([128, E - ES, 128]),
                op=OP.mult)

            # ---- attn @ A2 into p_t ----
            for ns in (slice(0, 512), slice(512, H)):
                nc.tensor.matmul(p_t[:, ns], attnT[:], a2b[:, ns],
                                 start=False, stop=False)

            # ---- D-reduce: 7-level TT tree over innermost j (2x) ----
            tDs = tpool.tile([128, E, 64], BF16, tag="tDs")
            dvb = spool.tile([128, E], BF16, tag="dvb")
            nc.vector.tensor_tensor(tDs[:, 0:ES, :], tmpD[:, 0:ES, 0:64],
                                    tmpD[:, 0:ES, 64:128], op=OP.add)
            nc.vector.tensor_tensor(tmpD[:, 0:ES, 0:32], tDs[:, 0:ES, 0:32],
                                    tDs[:, 0:ES, 32:64], op=OP.add)
            nc.vector.tensor_tensor(tDs[:, 0:ES, 0:16], tmpD[:, 0:ES, 0:16],
                                    tmpD[:, 0:ES, 16:32], op=OP.add)
            nc.vector.tensor_tensor(tmpD[:, 0:ES, 0:8], tDs[:, 0:ES, 0:8],
                                    tDs[:, 0:ES, 8:16], op=OP.add)
            nc.vector.tensor_tensor(tDs[:, 0:ES, 0:4], tmpD[:, 0:ES, 0:4],
                                    tmpD[:, 0:ES, 4:8], op=OP.add)
            nc.vector.tensor_tensor(tmpD[:, 0:ES, 0:2], tDs[:, 0:ES, 0:2],
                                    tDs[:, 0:ES, 2:4], op=OP.add)
            nc.vector.tensor_tensor(dvb[:, 0:ES], tmpD[:, 0:ES, 0:1],
                                    tmpD[:, 0:ES, 1:2], op=OP.add)
            nc.gpsimd.tensor_tensor(tDs[:, ES:E, :], tmpD[:, ES:E, 0:64],
                                    tmpD[:, ES:E, 64:128], op=OP.add)
            nc.gpsimd.tensor_tensor(tmpD[:, ES:E, 0:32], tDs[:, ES:E, 0:32],
                                    tDs[:, ES:E, 32:64], op=OP.add)
            nc.gpsimd.tensor_tensor(tDs[:, ES:E, 0:16], tmpD[:, ES:E, 0:16],
                                    tmpD[:, ES:E, 16:32], op=OP.add)
            nc.gpsimd.tensor_tensor(tmpD[:, ES:E, 0:8], tDs[:, ES:E, 0:8],
                                    tDs[:, ES:E, 8:16], op=OP.add)
            nc.gpsimd.tensor_tensor(tDs[:, ES:E, 0:4], tmpD[:, ES:E, 0:4],
                                    tmpD[:, ES:E, 4:8], op=OP.add)
            nc.gpsimd.tensor_tensor(tmpD[:, ES:E, 0:2], tDs[:, ES:E, 0:2],
                                    tDs[:, ES:E, 2:4], op=OP.add)
            nc.gpsimd.tensor_tensor(dvb[:, ES:E], tmpD[:, ES:E, 0:1],
                                    tmpD[:, ES:E, 1:2], op=OP.add)
            if b == 0:
                dump("d_dvec", dvb[:])

            st.update(ptb=ptb, p_t=p_t, dvb=dvb)
            return st

        def emit_back(b, st):
            ptb, p_t, dvb = st["ptb"], st["p_t"], st["dvb"]
            # D^T via PE transpose
            nc.tensor.transpose(ptb[0:E, 128:256], dvb[:], id_bf[:])
            dT = spool.tile([E, 128], BF16, tag="dT")
            nc.scalar.copy(dT[:], ptb[0:E, 128:256])
            # temp final: += D @ G1
            for ns in (slice(0, 512), slice(512, H)):
                nc.tensor.matmul(p_t[:, ns], dT[:], g1[:, ns],
                                 start=False, stop=True)
            # upd mask column
            nc.tensor.matmul(p_t[:, 1022:1023], vrow4[0:1, b, :],
                             ones_b[0:1, 0:1], start=True, stop=True)
            if b == 0:
                dump("d_upd", p_t[:, 1022:1023])
            # blend: out = upd*(temp - bertS) + bertS; rolled store
            outt = opool.tile([128, H], F32, tag="outt")
            nc.vector.scalar_tensor_tensor(
                outt[:], p_t[:, 0:H], p_t[:, 1022:1023], st["bertS"][:],
                op0=OP.mult, op1=OP.add)
            nc.sync.dma_start(out[b, 1:128, :], outt[0:127, :])
            nc.sync.dma_start(out[b, 0:1, :], outt[127:128, :])

        sts = {}
        for b in range(PB):
            sts[b] = emit_front(b)
            if b >= 1:
                emit_back(b - 1, sts.pop(b - 1))
        emit_back(PB - 1, sts.pop(PB - 1))


def _get_nc():
    if "nc" not in _CACHED:
        _CACHED["nc"] = _build(debug=bool(_CACHED.get("debug")))
    return _CACHED["nc"]


def _chunkT(w):
    """W [rows, K] -> W^T chunk-major [128, K//128, rows] (lhsT layout)."""
    rows, k = w.shape
    return np.ascontiguousarray(
        w.T.reshape(k // 128, 128, rows).transpose(1, 0, 2))


def _prep_in_maps(bert_hidden_states, dep_type_adj, deprel_adj,
                  asp_start, asp_end, Wz, bz, wa, ba, Wf, Wh):
    bf = ml_dtypes.bfloat16
    bert = np.ascontiguousarray(np.asarray(bert_hidden_states, np.float32))
    wa_f = np.asarray(wa, np.float32)
    wa_i, wa_j, wae_f = wa_f[:H], wa_f[H:2 * H], wa_f[2 * H:]
    wae_safe = np.where(wae_f == 0.0, 1.0, wae_f)
    # dep': wa_e folded in, transposed to [b, i, e, j]
    depW = np.asarray(dep_type_adj, np.float32) * wae_f[None, None, None, :]
    dept = np.ascontiguousarray(depW.transpose(0, 1, 3, 2)).astype(bf)
    adjn = np.asarray(deprel_adj) > 0
    madd = np.where(adjn, np.float32(0.0), np.float32(MASK_NEG))
    maddT = np.ascontiguousarray(madd.transpose(0, 2, 1)).astype(bf)
    # bertS^T chunk-major per batch: rows shifted by one (the z-roll)
    bs = np.ascontiguousarray(np.roll(bert, -1, axis=1))
    bertsT = np.ascontiguousarray(
        bs.transpose(0, 2, 1).reshape(B, KC, 128, L).transpose(0, 2, 1, 3)
    ).astype(bf)
    pos = np.arange(L, dtype=np.float32)
    s_ = np.asarray(asp_start).astype(np.float32)[:, None]
    e_ = np.asarray(asp_end).astype(np.float32)[:, None]
    vrow_full = (((pos[None, :] >= s_) & (pos[None, :] <= e_))
                 & adjn.any(-1)).astype(bf)

    Wz = np.asarray(Wz, np.float32)
    bz_f = np.asarray(bz, np.float32)
    ba_f = np.float32(np.asarray(ba, np.float32))
    Wf = np.asarray(Wf, np.float32)
    Wh = np.asarray(Wh, np.float32)
    WfZ, WfE = Wf[:, :H], Wf[:, H:]
    WhN, WhZ = Wh[:, :H], Wh[:, H:]
    G0 = WhN @ WfZ
    g0wT = _chunkT(G0 @ Wz).astype(bf)
    m1T = _chunkT(WhZ @ Wz - np.eye(H, dtype=np.float32)).astype(bf)
    g1 = np.ascontiguousarray(
        (WhN @ WfE).T / wae_safe[:, None]).astype(bf)
    u2 = np.stack([Wz.T @ wa_i, Wz.T @ wa_j], axis=0)  # [2, H]
    u2T = _chunkT(u2).astype(bf)
    brow = (WhZ @ bz_f + G0 @ bz_f)[None, :].astype(bf)
    bab = np.float32(ba_f + wa_i @ bz_f + wa_j @ bz_f).reshape(1, 1)

    in_maps = []
    for c in range(NCORES):
        s = slice(c * PB, (c + 1) * PB)
        in_maps.append(dict(
            berts=bs[s], bertsT=np.ascontiguousarray(bertsT[s]),
            dept=dept[s], maddT=maddT[s],
            vrow=np.ascontiguousarray(vrow_full[s][None, :, :]),
            g0wT=g0wT, m1T=m1T, g1=g1, u2T=u2T,
            browt=brow, bat=bab,
        ))
    return in_maps


def kernel(bert_hidden_states, dep_type_adj, deprel_adj, asp_start, asp_end,
           Wz, bz, wa, ba, Wf, Wh):
    from concourse.bass_utils import run_bass_kernel_spmd

    in_maps = _prep_in_maps(bert_hidden_states, dep_type_adj, deprel_adj,
                            asp_start, asp_end, Wz, bz, wa, ba, Wf, Wh)
    nc = _get_nc()
    res = run_bass_kernel_spmd(nc, in_maps, core_ids=list(range(NCORES)),
                               trace=bool(_CACHED.get("trace")),
                               tmpdir=_CACHED.get("trace_tmpdir"))
    _CACHED["last_results"] = res
    outs = [res.results[c]["out"] for c in range(NCORES)]
    return np.concatenate(outs, axis=0).astype(np.float32)


# revision 20
# speedup vs baseline: 1.1487x; 1.0628x over previous
"""Trainium2 Bass kernel for AspectNeighborAttention (gnn_message_passing).

Pure data-parallel over batch: 32 batches -> 8 NeuronCores x 4 batches.
All weights replicated, host-converted to bf16 and host-PRE-TRANSPOSED into
the chunk-major [128, KC, *] lhsT/rhs layouts the TensorEngine wants.

v2 redesign (from v1 at ~167us measured):
  * The zs GEMM is gone entirely: every consumer of zs is host-folded onto
    bertS directly (A2 = bertS @ (G0 Wz)^T, temp1 = bertS @ (WhZ Wz - I)^T
    which also folds the -bertS blend term, s_i/s_j = bertS @ (Wz^T wa_*)
    with the bz constants folded into ba / a bias row).  PE instruction
    count halves and the serial zsT dependency disappears.
  * dep is host-transposed to [i, e, j] (wa_e pre-folded, bf16; 1/wa_e
    folded into G1 rows so D' @ G1' == D @ G1 exactly):
      - s_e = sum_e dep': binary TT-add tree over the MIDDLE e axis.
        TensorReduce has NO DVE fast modes (1.04 ns/elem always) but
        TensorTensor has 2x_1p (0.52 ns/elem when every operand is 2-byte
        with innermost stride 1), so a 6-level tree (8064 elems) beats the
        single reduce (8192 elems) 2:1.
      - D-mult tmp = dep' * attn[i,j]-broadcast: the broadcast is over the
        middle axis so the innermost stride stays 1 -> 2x mode (v1's
        innermost-stride-0 broadcast forced 1x).
      - D-reduce over the innermost j axis: 7-level TT tree at 2x.
    DVE work per batch drops ~22us -> ~12us (cost-model validated).
  * Masking is additive and PE-folded: host sends madd^T (0 / -1e4), a
    maddT x Identity matmul accumulates it into the same PSUM tile as the
    s_j row broadcast, so score = lrelu(se + si_col + sjmadd) needs only
    2 DVE STTs; masked entries hit exp underflow -> exact 0 attn.  The
    row-max pass is dropped (scores are bounded ~+-8, exp is safe); sumex
    gets +1e-30 so all-masked rows yield attn=0 instead of NaN.
  * attn = ex * rec as a bf16 tensor_scalar (4x_2p mode, ~33ns).
  * GpSimd carries a balanced slice of each tree level + the e-tail of the
    D path; emission is software-pipelined (back(b-1) emitted after
    front(b)) so dT/G1/blend of batch b-1 never head-of-line-block batch
    b's PE/DVE front-end work.  PSUM: 3x p_big(2 banks) + 2x p_x(1) = 8.
"""

import sys

for _p in ("/opt/trn_rl_repo",):
    if _p not in sys.path:
        sys.path.insert(0, _p)

import os
import numpy as np
import ml_dtypes

import concourse.bass as bass
import concourse.bacc as bacc_mod
import concourse.mybir as mybir
import concourse.tile as tile
from concourse.masks import make_identity

B, L, H, E = 32, 128, 768, 64
NCORES = 8
PB = B // NCORES  # batches per core
KC = H // 128     # 6 k-chunks
F32 = mybir.dt.float32
BF16 = mybir.dt.bfloat16
AF = mybir.ActivationFunctionType
OP = mybir.AluOpType
AX = mybir.AxisListType
MASK_NEG = -10000.0

_CACHED = {}

CFG = dict(
    dep_bufs=int(os.environ.get("K_DEP_BUFS", 3)),
    tmpd_bufs=int(os.environ.get("K_TMPD_BUFS", 2)),
    spool_bufs=int(os.environ.get("K_SPOOL_BUFS", 3)),
    opool_bufs=int(os.environ.get("K_OPOOL_BUFS", 2)),
    px_bufs=int(os.environ.get("K_PX_BUFS", 2)),
    pbig_bufs=int(os.environ.get("K_PBIG_BUFS", 2)),
    emv=int(os.environ.get("K_EMV", 48)),   # D-mult: e[0:emv) DVE, rest GpSimd
    edr=int(os.environ.get("K_EDR", 40)),   # D-reduce: e[0:edr) DVE, rest GpSimd
    jsplit=int(os.environ.get("K_JSPLIT", 96)),  # se L1/L2: j[0:js) DVE, rest GpSimd
    gblend=int(os.environ.get("K_GBLEND", 0)),  # 1: blend on GpSimd via ACT
    # staging (STT not in Pool ISA -> keep 0 = DVE)
)


def _build(debug=False):
    nc = bacc_mod.Bacc("TRN2", target_bir_lowering=False, debug=False,
                       num_devices=NCORES)

    bert = nc.dram_tensor("berts", [PB, L, H], F32, kind="ExternalInput")
    bertsT = nc.dram_tensor("bertsT", [PB, 128, KC, 128], BF16,
                            kind="ExternalInput")
    dept_d = nc.dram_tensor("dept", [PB, 128, E, 128], BF16,
                            kind="ExternalInput")
    maddT_d = nc.dram_tensor("maddT", [PB, 128, 128], BF16,
                             kind="ExternalInput")
    vrow = nc.dram_tensor("vrow", [1, PB, 128], BF16, kind="ExternalInput")
    g0wT_d = nc.dram_tensor("g0wT", [128, KC, H], BF16, kind="ExternalInput")
    m1T_d = nc.dram_tensor("m1T", [128, KC, H], BF16, kind="ExternalInput")
    g1_d = nc.dram_tensor("g1", [E, H], BF16, kind="ExternalInput")
    u2T_d = nc.dram_tensor("u2T", [128, KC, 2], BF16, kind="ExternalInput")
    browt = nc.dram_tensor("browt", [1, H], BF16, kind="ExternalInput")
    bat = nc.dram_tensor("bat", [1, 1], F32, kind="ExternalInput")
    out = nc.dram_tensor("out", [PB, L, H], F32, kind="ExternalOutput")

    dbg = {}
    if debug:
        for nm, shape, dt in [
            ("d_si", [1, 128], F32), ("d_sjb", [1, 128], F32),
            ("d_se", [128, L], BF16), ("d_sadd", [128, L], F32),
            ("d_attn", [128, L], BF16), ("d_dvec", [128, E], BF16),
            ("d_a2b", [128, H], BF16), ("d_upd", [128, 1], F32),
        ]:
            dbg[nm] = nc.dram_tensor(nm, shape, dt, kind="ExternalOutput")
    with tile.TileContext(nc) as tc:
        with nc.allow_low_precision("bf16 softmax/D path, 2e-2 rel-err gate"):
            _body(tc, nc, bert, bertsT, dept_d, maddT_d, vrow, g0wT_d, m1T_d,
                  g1_d, u2T_d, browt, bat, out, dbg)
    nc.compile()
    return nc


def _body(tc, nc, bert, bertsT, dept_d, maddT_d, vrow, g0wT_d, m1T_d,
          g1_d, u2T_d, browt, bat, out, dbg=None):
    def dump(name, ap):
        if dbg and name in dbg:
            nc.sync.dma_start(dbg[name][...], ap)
    import contextlib
    cfg = CFG
    EMV = cfg["emv"]
    EDR = cfg["edr"]
    JS = cfg["jsplit"]
    ctx = contextlib.ExitStack()
    with ctx:
        wpool = ctx.enter_context(tc.tile_pool(name="weights", bufs=1))
        dpool = ctx.enter_context(
            tc.tile_pool(name="dep", bufs=cfg["dep_bufs"]))
        tpool = ctx.enter_context(
            tc.tile_pool(name="tmpd", bufs=cfg["tmpd_bufs"]))
        spool = ctx.enter_context(
            tc.tile_pool(name="small", bufs=cfg["spool_bufs"]))
        opool = ctx.enter_context(
            tc.tile_pool(name="outp", bufs=cfg["opool_bufs"]))
        # PSUM budget (8 banks): p_t [128,1024]f32 = 2 banks x2 bufs,
        # p_a [128,512]f32 = 1 bank x2, ptb [128,256]bf16 = 1 bank x2.
        p_apool = ctx.enter_context(
            tc.tile_pool(name="p_a", bufs=cfg["px_bufs"], space="PSUM"))
        p_tb = ctx.enter_context(
            tc.tile_pool(name="p_tb", bufs=cfg["px_bufs"], space="PSUM"))
        p_big = ctx.enter_context(
            tc.tile_pool(name="p_big", bufs=cfg["pbig_bufs"], space="PSUM"))

        # ---------------- input-batch prefetch (emitted FIRST so batch-0
        # dep isn't queued behind 2.4MB of weights) ----------------
        def prefetch(b):
            st = {}
            dept = dpool.tile([128, E, 128], BF16, tag="dept")
            nc.sync.dma_start(dept[:], dept_d[b, :, :, :])
            bertST = spool.tile([128, KC, 128], BF16, tag="bertST")
            nc.sync.dma_start(bertST[:], bertsT[b, :, :, :])
            maddT = spool.tile([128, 128], BF16, tag="maddT")
            nc.sync.dma_start(maddT[:], maddT_d[b, :, :])
            bertS = spool.tile([128, H], F32, tag="bertS")
            nc.sync.dma_start(bertS[:], bert[b, :, :])
            st.update(bertS=bertS, dept=dept, bertST=bertST, maddT=maddT)
            return st

        st0 = prefetch(0)

        # ---------------- one-time setup (plain DMAs only) ----------------
        g0wT = wpool.tile([128, KC, H], BF16, tag="g0wT")
        nc.sync.dma_start(g0wT[:], g0wT_d[...])
        m1T = wpool.tile([128, KC, H], BF16, tag="m1T")
        nc.sync.dma_start(m1T[:], m1T_d[...])
        g1 = wpool.tile([E, H], BF16, tag="g1")
        nc.sync.dma_start(g1[:], g1_d[...])
        u2T = wpool.tile([128, KC, 2], BF16, tag="u2T")
        nc.sync.dma_start(u2T[:], u2T_d[...])
        brow = wpool.tile([1, H], BF16, tag="brow")
        nc.sync.dma_start(brow[:], browt[:, :])
        bar = wpool.tile([1, 1], F32, tag="bar")
        nc.sync.dma_start(bar[:], bat[:, :])
        vrow4 = wpool.tile([1, PB, 128], BF16, tag="vrow4")
        nc.sync.dma_start(vrow4[:], vrow[:, :, :])

        ones_f = wpool.tile([1, 128], F32, tag="ones_f")
        nc.gpsimd.memset(ones_f[:], 1.0)
        ones_b = wpool.tile([1, 128], BF16, tag="ones_b")
        nc.gpsimd.memset(ones_b[:], 1.0)
        id_bf = wpool.tile([128, 128], BF16, tag="id_bf")
        make_identity(nc, id_bf[:])

        # -------- per-batch pipeline --------
        def emit_front(b, st):
            dept, bertST, maddT = st["dept"], st["bertST"], st["maddT"]

            # ---- A2 = bertS @ (G0 Wz)^T (two chunks through a 1-bank ring) --
            a2b = spool.tile([128, H], BF16, tag="a2b")
            for ns in (slice(0, 512), slice(512, H)):
                p_a = p_apool.tile([128, 512], F32, tag="p_a")
                w = ns.stop - ns.start
                for kc in range(KC):
                    nc.tensor.matmul(p_a[:, 0:w], bertST[:, kc, :],
                                     g0wT[:, kc, ns],
                                     start=(kc == 0), stop=(kc == KC - 1))
                nc.scalar.copy(a2b[:, ns], p_a[:, 0:w])
            if b == 0:
                dump("d_a2b", a2b[:])

            # p_t [128,1024] = 2 banks: [0:768] temp accum; [768:896] s_i row
            # then (WAR) sj+madd bcast; [896:1024] s_j row then (WAR) si col
            # at 1023 and upd col at 1022.
            p_t = p_big.tile([128, 1024], F32, tag="p_big")

            # ---- s_i / s_j rows (m=1 each; DVE can't read partition 1) ----
            for kc in range(KC):
                nc.tensor.matmul(p_t[0:1, 768:896], u2T[:, kc, 0:1],
                                 bertST[:, kc, :],
                                 start=(kc == 0), stop=(kc == KC - 1))
            for kc in range(KC):
                nc.tensor.matmul(p_t[0:1, 896:1024], u2T[:, kc, 1:2],
                                 bertST[:, kc, :],
                                 start=(kc == 0), stop=(kc == KC - 1))
            si_row = spool.tile([1, 128], F32, tag="si_row")
            nc.scalar.copy(si_row[:], p_t[0:1, 768:896])
            sjb = spool.tile([1, 128], F32, tag="sjb")
            nc.vector.tensor_scalar(sjb[:], p_t[0:1, 896:1024], bar[0:1, 0:1],
                                    None, op0=OP.add)
            # sj row bcast + additive mask (WAR over s_i region), si col
            nc.tensor.matmul(p_t[:, 768:896], maddT[:], id_bf[:],
                             start=True, stop=False)
            nc.tensor.matmul(p_t[:, 768:896], ones_f[:], sjb[:],
                             start=False, stop=True)
            nc.tensor.matmul(p_t[:, 1023:1024], si_row[:], ones_f[0:1, 0:1],
                             start=True, stop=True)
            if b == 0:
                dump("d_si", si_row[:])
                dump("d_sjb", sjb[:])

            # ---- temp1 = bertS @ (WhZ Wz - I)^T + brow ----
            for ns in (slice(0, 512), slice(512, H)):
                for kc in range(KC):
                    nc.tensor.matmul(p_t[:, ns], bertST[:, kc, :],
                                     m1T[:, kc, ns],
                                     start=(kc == 0), stop=False)
                nc.tensor.matmul(p_t[:, ns], ones_b[:], brow[0:1, ns],
                                 start=False, stop=False)

            # ---- s_e: 6-level TT tree over middle e axis (2x mode) ----
            seA = tpool.tile([128, 32, 128], BF16, tag="seA")
            seB = tpool.tile([128, 16, 128], BF16, tag="seB")
            nc.vector.tensor_tensor(seA[:, :, 0:JS], dept[:, 0:32, 0:JS],
                                    dept[:, 32:64, 0:JS], op=OP.add)
            nc.gpsimd.tensor_tensor(seA[:, :, JS:128], dept[:, 0:32, JS:128],
                                    dept[:, 32:64, JS:128], op=OP.add)
            nc.vector.tensor_tensor(seB[:, :, 0:JS], seA[:, 0:16, 0:JS],
                                    seA[:, 16:32, 0:JS], op=OP.add)
            nc.gpsimd.tensor_tensor(seB[:, :, JS:128], seA[:, 0:16, JS:128],
                                    seA[:, 16:32, JS:128], op=OP.add)
            nc.vector.tensor_tensor(seA[:, 0:8, :], seB[:, 0:8, :],
                                    seB[:, 8:16, :], op=OP.add)
            nc.vector.tensor_tensor(seB[:, 0:4, :], seA[:, 0:4, :],
                                    seA[:, 4:8, :], op=OP.add)
            nc.vector.tensor_tensor(seA[:, 0:2, :], seB[:, 0:2, :],
                                    seB[:, 2:4, :], op=OP.add)
            sef = spool.tile([128, 128], BF16, tag="sef")
            nc.vector.tensor_tensor(sef[:], seA[:, 0, :], seA[:, 1, :],
                                    op=OP.add)
            if b == 0:
                dump("d_se", sef[:])

            # ---- score = lrelu(se + si + sj + madd); softmax (no rowmax) ----
            sadd = spool.tile([128, L], F32, tag="sadd")
            nc.vector.scalar_tensor_tensor(
                sadd[:], sef[:], p_t[:, 1023:1024], p_t[:, 768:896],
                op0=OP.add, op1=OP.add)
            score = spool.tile([128, L], F32, tag="score")
            nc.vector.scalar_tensor_tensor(
                score[:], sadd[:], 0.01, sadd[:], op0=OP.mult, op1=OP.max)
            if b == 0:
                dump("d_sadd", score[:])
            ex = spool.tile([128, L], BF16, tag="ex")
            sumex = spool.tile([128, 1], F32, tag="sumex")
            nc.scalar.activation(ex[:], score[:], AF.Exp, bias=0.0,
                                 scale=1.0, accum_out=sumex[:])
            sume = spool.tile([128, 1], F32, tag="sume")
            nc.vector.tensor_scalar(sume[:], sumex[:], 1e-30, None,
                                    op0=OP.add)
            rec = spool.tile([128, 1], F32, tag="rec")
            nc.vector.reciprocal(rec[:], sume[:])
            attnb = spool.tile([128, L], BF16, tag="attnb")
            nc.vector.tensor_scalar(attnb[:], ex[:], rec[0:128, 0:1], None,
                                    op0=OP.mult)
            if b == 0:
                dump("d_attn", attnb[:])

            # attn^T via PE (bf16 PSUM ring shared with the dT transpose)
            ptb = p_tb.tile([128, 256], BF16, tag="p_tb")
            nc.tensor.transpose(ptb[:, 0:128], attnb[:], id_bf[:])
            attnT = spool.tile([128, 128], BF16, tag="attnT")
            nc.scalar.copy(attnT[:], ptb[:, 0:128])

            # ---- D-mult: tmp[i,e,j] = dep'[i,e,j] * attn[i,j] ----
            # GpSimd's share reads an ACT-materialized broadcast (Pool's
            # stride-0 reads measured ~3x slower than its contiguous ones).
            tmpD = tpool.tile([128, E, 128], BF16, tag="tmpD")
            attnW = tpool.tile([128, E - EMV, 128], BF16, tag="attnW")
            nc.scalar.copy(
                attnW[:],
                attnb[:].unsqueeze(1).broadcast_to([128, E - EMV, 128]))
            nc.vector.tensor_tensor(
                tmpD[:, 0:EMV, :], dept[:, 0:EMV, :],
                attnb[:].unsqueeze(1).broadcast_to([128, EMV, 128]),
                op=OP.mult)
            nc.gpsimd.tensor_tensor(
                tmpD[:, EMV:E, :], dept[:, EMV:E, :], attnW[:], op=OP.mult)

            # ---- attn @ A2 into p_t ----
            for ns in (slice(0, 512), slice(512, H)):
                nc.tensor.matmul(p_t[:, ns], attnT[:], a2b[:, ns],
                                 start=False, stop=False)

            # ---- D-reduce over innermost j: DVE single reduce for the head,
            # GpSimd TT tree for the e-tail (Pool has no free-axis reduce) --
            dvb = spool.tile([128, E], BF16, tag="dvb")
            nc.vector.tensor_reduce(dvb[:, 0:EDR], tmpD[:, 0:EDR, :],
                                    axis=AX.X, op=OP.add)
            tDs = tpool.tile([128, E - EDR, 64], BF16, tag="tDs")
            nc.gpsimd.tensor_tensor(tDs[:, :, :], tmpD[:, EDR:E, 0:64],
                                    tmpD[:, EDR:E, 64:128], op=OP.add)
            nc.gpsimd.tensor_tensor(tmpD[:, EDR:E, 0:32], tDs[:, :, 0:32],
                                    tDs[:, :, 32:64], op=OP.add)
            nc.gpsimd.tensor_tensor(tDs[:, :, 0:16], tmpD[:, EDR:E, 0:16],
                                    tmpD[:, EDR:E, 16:32], op=OP.add)
            nc.gpsimd.tensor_tensor(tmpD[:, EDR:E, 0:8], tDs[:, :, 0:8],
                                    tDs[:, :, 8:16], op=OP.add)
            nc.gpsimd.tensor_tensor(tDs[:, :, 0:4], tmpD[:, EDR:E, 0:4],
                                    tmpD[:, EDR:E, 4:8], op=OP.add)
            nc.gpsimd.tensor_tensor(tmpD[:, EDR:E, 0:2], tDs[:, :, 0:2],
                                    tDs[:, :, 2:4], op=OP.add)
            nc.gpsimd.tensor_tensor(dvb[:, EDR:E], tmpD[:, EDR:E, 0:1],
                                    tmpD[:, EDR:E, 1:2], op=OP.add)
            if b == 0:
                dump("d_dvec", dvb[:])

            st.update(ptb=ptb, p_t=p_t, dvb=dvb)
            return st

        def emit_back(b, st):
            ptb, p_t, dvb = st["ptb"], st["p_t"], st["dvb"]
            # D^T via PE transpose
            nc.tensor.transpose(ptb[0:E, 128:256], dvb[:], id_bf[:])
            dT = spool.tile([E, 128], BF16, tag="dT")
            nc.scalar.copy(dT[:], ptb[0:E, 128:256])
            # temp final: += D @ G1
            for ns in (slice(0, 512), slice(512, H)):
                nc.tensor.matmul(p_t[:, ns], dT[:], g1[:, ns],
                                 start=False, stop=True)
            # upd mask column
            nc.tensor.matmul(p_t[:, 1022:1023], vrow4[0:1, b, :],
                             ones_b[0:1, 0:1], start=True, stop=True)
            if b == 0:
                dump("d_upd", p_t[:, 1022:1023])
            # blend: out = upd*(temp - bertS) + bertS; rolled store.
            # GpSimd can't read PSUM, so ACT stages temp+upd into SBUF first.
            outt = opool.tile([128, H], F32, tag="outt")
            if cfg["gblend"]:
                tsb = opool.tile([128, H], F32, tag="tsb")
                nc.scalar.copy(tsb[:], p_t[:, 0:H])
                upds = spool.tile([128, 1], F32, tag="upds")
                nc.scalar.copy(upds[:], p_t[:, 1022:1023])
                nc.gpsimd.scalar_tensor_tensor(
                    outt[:], tsb[:], upds[0:128, 0:1], st["bertS"][:],
                    op0=OP.mult, op1=OP.add)
            else:
                nc.vector.scalar_tensor_tensor(
                    outt[:], p_t[:, 0:H], p_t[:, 1022:1023], st["bertS"][:],
                    op0=OP.mult, op1=OP.add)
            nc.sync.dma_start(out[b, 1:128, :], outt[0:127, :])
            nc.sync.dma_start(out[b, 0:1, :], outt[127:128, :])

        sts = {0: st0}
        for b in range(PB):
            sts[b] = emit_front(b, sts[b])
            if b + 1 < PB:
                sts[b + 1] = prefetch(b + 1)
            if b >= 1:
                emit_back(b - 1, sts.pop(b - 1))
        emit_back(PB - 1, sts.pop(PB - 1))


def _get_nc():
    if "nc" not in _CACHED:
        _CACHED["nc"] = _build(debug=bool(_CACHED.get("debug")))
    return _CACHED["nc"]


def _chunkT(w):
    """W [rows, K] -> W^T chunk-major [128, K//128, rows] (lhsT layout)."""
    rows, k = w.shape
    return np.ascontiguousarray(
        w.T.reshape(k // 128, 128, rows).transpose(1, 0, 2))


def _prep_in_maps(bert_hidden_states, dep_type_adj, deprel_adj,
                  asp_start, asp_end, Wz, bz, wa, ba, Wf, Wh):
    bf = ml_dtypes.bfloat16
    bert = np.ascontiguousarray(np.asarray(bert_hidden_states, np.float32))
    wa_f = np.asarray(wa, np.float32)
    wa_i, wa_j, wae_f = wa_f[:H], wa_f[H:2 * H], wa_f[2 * H:]
    wae_safe = np.where(wae_f == 0.0, 1.0, wae_f)
    # dep': wa_e folded in, transposed to [b, i, e, j]
    depW = np.asarray(dep_type_adj, np.float32) * wae_f[None, None, None, :]
    dept = np.ascontiguousarray(depW.transpose(0, 1, 3, 2)).astype(bf)
    adjn = np.asarray(deprel_adj) > 0
    madd = np.where(adjn, np.float32(0.0), np.float32(MASK_NEG))
    maddT = np.ascontiguousarray(madd.transpose(0, 2, 1)).astype(bf)
    # bertS^T chunk-major per batch: rows shifted by one (the z-roll)
    bs = np.ascontiguousarray(np.roll(bert, -1, axis=1))
    bertsT = np.ascontiguousarray(
        bs.transpose(0, 2, 1).reshape(B, KC, 128, L).transpose(0, 2, 1, 3)
    ).astype(bf)
    pos = np.arange(L, dtype=np.float32)
    s_ = np.asarray(asp_start).astype(np.float32)[:, None]
    e_ = np.asarray(asp_end).astype(np.float32)[:, None]
    vrow_full = (((pos[None, :] >= s_) & (pos[None, :] <= e_))
                 & adjn.any(-1)).astype(bf)

    Wz = np.asarray(Wz, np.float32)
    bz_f = np.asarray(bz, np.float32)
    ba_f = np.float32(np.asarray(ba, np.float32))
    Wf = np.asarray(Wf, np.float32)
    Wh = np.asarray(Wh, np.float32)
    WfZ, WfE = Wf[:, :H], Wf[:, H:]
    WhN, WhZ = Wh[:, :H], Wh[:, H:]
    G0 = WhN @ WfZ
    g0wT = _chunkT(G0 @ Wz).astype(bf)
    m1T = _chunkT(WhZ @ Wz - np.eye(H, dtype=np.float32)).astype(bf)
    g1 = np.ascontiguousarray(
        (WhN @ WfE).T / wae_safe[:, None]).astype(bf)
    u2 = np.stack([Wz.T @ wa_i, Wz.T @ wa_j], axis=0)  # [2, H]
    u2T = _chunkT(u2).astype(bf)
    brow = (WhZ @ bz_f + G0 @ bz_f)[None, :].astype(bf)
    bab = np.float32(ba_f + wa_i @ bz_f + wa_j @ bz_f).reshape(1, 1)

    in_maps = []
    for c in range(NCORES):
        s = slice(c * PB, (c + 1) * PB)
        in_maps.append(dict(
            berts=bs[s], bertsT=np.ascontiguousarray(bertsT[s]),
            dept=dept[s], maddT=maddT[s],
            vrow=np.ascontiguousarray(vrow_full[s][None, :, :]),
            g0wT=g0wT, m1T=m1T, g1=g1, u2T=u2T,
            browt=brow, bat=bab,
        ))
    return in_maps


def kernel(bert_hidden_states, dep_type_adj, deprel_adj, asp_start, asp_end,
           Wz, bz, wa, ba, Wf, Wh):
    from concourse.bass_utils import run_bass_kernel_spmd

    in_maps = _prep_in_maps(bert_hidden_states, dep_type_adj, deprel_adj,
                            asp_start, asp_end, Wz, bz, wa, ba, Wf, Wh)
    nc = _get_nc()
    res = run_bass_kernel_spmd(nc, in_maps, core_ids=list(range(NCORES)),
                               trace=bool(_CACHED.get("trace")),
                               tmpdir=_CACHED.get("trace_tmpdir"))
    _CACHED["last_results"] = res
    outs = [res.results[c]["out"] for c in range(NCORES)]
    return np.concatenate(outs, axis=0).astype(np.float32)


# revision 24
# speedup vs baseline: 1.2262x; 1.0675x over previous
"""Trainium2 Bass kernel for AspectNeighborAttention (gnn_message_passing).

Pure data-parallel over batch: 32 batches -> 8 NeuronCores x 4 batches.
All weights replicated, host-converted to bf16 and host-PRE-TRANSPOSED into
the chunk-major [128, KC, *] lhsT/rhs layouts the TensorEngine wants.

v2 redesign (from v1 at ~167us measured):
  * The zs GEMM is gone entirely: every consumer of zs is host-folded onto
    bertS directly (A2 = bertS @ (G0 Wz)^T, temp1 = bertS @ (WhZ Wz - I)^T
    which also folds the -bertS blend term, s_i/s_j = bertS @ (Wz^T wa_*)
    with the bz constants folded into ba / a bias row).  PE instruction
    count halves and the serial zsT dependency disappears.
  * dep is host-transposed to [i, e, j] (wa_e pre-folded, bf16; 1/wa_e
    folded into G1 rows so D' @ G1' == D @ G1 exactly):
      - s_e = sum_e dep': binary TT-add tree over the MIDDLE e axis.
        TensorReduce has NO DVE fast modes (1.04 ns/elem always) but
        TensorTensor has 2x_1p (0.52 ns/elem when every operand is 2-byte
        with innermost stride 1), so a 6-level tree (8064 elems) beats the
        single reduce (8192 elems) 2:1.
      - D-mult tmp = dep' * attn[i,j]-broadcast: the broadcast is over the
        middle axis so the innermost stride stays 1 -> 2x mode (v1's
        innermost-stride-0 broadcast forced 1x).
      - D-reduce over the innermost j axis: 7-level TT tree at 2x.
    DVE work per batch drops ~22us -> ~12us (cost-model validated).
  * Masking is additive and PE-folded: host sends madd^T (0 / -1e4), a
    maddT x Identity matmul accumulates it into the same PSUM tile as the
    s_j row broadcast, so score = lrelu(se + si_col + sjmadd) needs only
    2 DVE STTs; masked entries hit exp underflow -> exact 0 attn.  The
    row-max pass is dropped (scores are bounded ~+-8, exp is safe); sumex
    gets +1e-30 so all-masked rows yield attn=0 instead of NaN.
  * attn = ex * rec as a bf16 tensor_scalar (4x_2p mode, ~33ns).
  * GpSimd carries a balanced slice of each tree level + the e-tail of the
    D path; emission is software-pipelined (back(b-1) emitted after
    front(b)) so dT/G1/blend of batch b-1 never head-of-line-block batch
    b's PE/DVE front-end work.  PSUM: 3x p_big(2 banks) + 2x p_x(1) = 8.
"""

import sys

for _p in ("/opt/trn_rl_repo",):
    if _p not in sys.path:
        sys.path.insert(0, _p)

import os
import numpy as np
import ml_dtypes

import concourse.bass as bass
import concourse.bacc as bacc_mod
import concourse.mybir as mybir
import concourse.tile as tile
from concourse.masks import make_identity

B, L, H, E = 32, 128, 768, 64
NCORES = 8
PB = B // NCORES  # batches per core
KC = H // 128     # 6 k-chunks
F32 = mybir.dt.float32
BF16 = mybir.dt.bfloat16
AF = mybir.ActivationFunctionType
OP = mybir.AluOpType
AX = mybir.AxisListType
MASK_NEG = -10000.0

_CACHED = {}

CFG = dict(
    dep_bufs=int(os.environ.get("K_DEP_BUFS", 3)),
    tmpd_bufs=int(os.environ.get("K_TMPD_BUFS", 2)),
    spool_bufs=int(os.environ.get("K_SPOOL_BUFS", 3)),
    opool_bufs=int(os.environ.get("K_OPOOL_BUFS", 2)),
    px_bufs=int(os.environ.get("K_PX_BUFS", 2)),
    pbig_bufs=int(os.environ.get("K_PBIG_BUFS", 2)),
    emv=int(os.environ.get("K_EMV", 56)),   # D-mult: e[0:emv) DVE, rest GpSimd
    edr=int(os.environ.get("K_EDR", 44)),   # D-reduce: e[0:edr) DVE, rest GpSimd
    l1g=int(os.environ.get("K_L1G", 24)),   # se L1: rows [0:l1g) DVE, rest GpSimd
    l2g=int(os.environ.get("K_L2G", 12)),   # se L2: rows [0:l2g) DVE, rest GpSimd
)


def _build(debug=False):
    nc = bacc_mod.Bacc("TRN2", target_bir_lowering=False, debug=False,
                       num_devices=NCORES)

    bert = nc.dram_tensor("berts", [PB, L, H], F32, kind="ExternalInput")
    bertsT = nc.dram_tensor("bertsT", [PB, 128, KC, 128], BF16,
                            kind="ExternalInput")
    dept_d = nc.dram_tensor("dept", [PB, 128, E, 128], BF16,
                            kind="ExternalInput")
    maddT_d = nc.dram_tensor("maddT", [PB, 128, 128], BF16,
                             kind="ExternalInput")
    vrow = nc.dram_tensor("vrow", [1, PB, 128], BF16, kind="ExternalInput")
    g0wT_d = nc.dram_tensor("g0wT", [128, KC, H], BF16, kind="ExternalInput")
    m1T_d = nc.dram_tensor("m1T", [128, KC, H], BF16, kind="ExternalInput")
    g1_d = nc.dram_tensor("g1", [E, H], BF16, kind="ExternalInput")
    u2T_d = nc.dram_tensor("u2T", [128, KC, 2], BF16, kind="ExternalInput")
    browt = nc.dram_tensor("browt", [1, H], BF16, kind="ExternalInput")
    bat = nc.dram_tensor("bat", [1, 1], F32, kind="ExternalInput")
    out = nc.dram_tensor("out", [PB, L, H], F32, kind="ExternalOutput")

    dbg = {}
    if debug:
        for nm, shape, dt in [
            ("d_si", [1, 128], F32), ("d_sjb", [1, 128], F32),
            ("d_se", [128, L], BF16), ("d_sadd", [128, L], F32),
            ("d_attn", [128, L], BF16), ("d_dvec", [128, E], BF16),
            ("d_a2b", [128, H], BF16), ("d_upd", [128, 1], F32),
        ]:
            dbg[nm] = nc.dram_tensor(nm, shape, dt, kind="ExternalOutput")
    with tile.TileContext(nc) as tc:
        with nc.allow_low_precision("bf16 softmax/D path, 2e-2 rel-err gate"):
            _body(tc, nc, bert, bertsT, dept_d, maddT_d, vrow, g0wT_d, m1T_d,
                  g1_d, u2T_d, browt, bat, out, dbg)
    nc.compile()
    return nc


def _body(tc, nc, bert, bertsT, dept_d, maddT_d, vrow, g0wT_d, m1T_d,
          g1_d, u2T_d, browt, bat, out, dbg=None):
    def dump(name, ap):
        if dbg and name in dbg:
            nc.sync.dma_start(dbg[name][...], ap)
    import contextlib
    cfg = CFG
    EMV = cfg["emv"]
    EDR = cfg["edr"]
    L1G = cfg["l1g"]
    L2G = cfg["l2g"]
    ctx = contextlib.ExitStack()
    with ctx:
        wpool = ctx.enter_context(tc.tile_pool(name="weights", bufs=1))
        dpool = ctx.enter_context(
            tc.tile_pool(name="dep", bufs=cfg["dep_bufs"]))
        tpool = ctx.enter_context(
            tc.tile_pool(name="tmpd", bufs=cfg["tmpd_bufs"]))
        spool = ctx.enter_context(
            tc.tile_pool(name="small", bufs=cfg["spool_bufs"]))
        opool = ctx.enter_context(
            tc.tile_pool(name="outp", bufs=cfg["opool_bufs"]))
        # PSUM budget (8 banks): p_t [128,1024]f32 = 2 banks x2 bufs,
        # p_a [128,512]f32 = 1 bank x2, ptb [128,256]bf16 = 1 bank x2.
        p_apool = ctx.enter_context(
            tc.tile_pool(name="p_a", bufs=cfg["px_bufs"], space="PSUM"))
        p_tb = ctx.enter_context(
            tc.tile_pool(name="p_tb", bufs=cfg["px_bufs"], space="PSUM"))
        p_big = ctx.enter_context(
            tc.tile_pool(name="p_big", bufs=cfg["pbig_bufs"], space="PSUM"))

        # ---------------- input-batch prefetch (emitted FIRST so batch-0
        # dep isn't queued behind 2.4MB of weights) ----------------
        def prefetch(b):
            st = {}
            dept = dpool.tile([128, E, 128], BF16, tag="dept")
            nc.sync.dma_start(dept[:], dept_d[b, :, :, :])
            bertST = spool.tile([128, KC, 128], BF16, tag="bertST")
            nc.sync.dma_start(bertST[:], bertsT[b, :, :, :])
            maddT = spool.tile([128, 128], BF16, tag="maddT")
            nc.sync.dma_start(maddT[:], maddT_d[b, :, :])
            bertS = spool.tile([128, H], F32, tag="bertS")
            nc.sync.dma_start(bertS[:], bert[b, :, :])
            st.update(bertS=bertS, dept=dept, bertST=bertST, maddT=maddT)
            return st

        st0 = prefetch(0)

        # ---------------- one-time setup (plain DMAs only) ----------------
        g0wT = wpool.tile([128, KC, H], BF16, tag="g0wT")
        nc.sync.dma_start(g0wT[:], g0wT_d[...])
        m1T = wpool.tile([128, KC, H], BF16, tag="m1T")
        nc.sync.dma_start(m1T[:], m1T_d[...])
        g1 = wpool.tile([E, H], BF16, tag="g1")
        nc.sync.dma_start(g1[:], g1_d[...])
        u2T = wpool.tile([128, KC, 2], BF16, tag="u2T")
        nc.sync.dma_start(u2T[:], u2T_d[...])
        brow = wpool.tile([1, H], BF16, tag="brow")
        nc.sync.dma_start(brow[:], browt[:, :])
        bar = wpool.tile([1, 1], F32, tag="bar")
        nc.sync.dma_start(bar[:], bat[:, :])
        vrow4 = wpool.tile([1, PB, 128], BF16, tag="vrow4")
        nc.sync.dma_start(vrow4[:], vrow[:, :, :])

        ones_f = wpool.tile([1, 128], F32, tag="ones_f")
        nc.gpsimd.memset(ones_f[:], 1.0)
        ones_b = wpool.tile([1, 128], BF16, tag="ones_b")
        nc.gpsimd.memset(ones_b[:], 1.0)
        id_bf = wpool.tile([128, 128], BF16, tag="id_bf")
        make_identity(nc, id_bf[:])

        # -------- per-batch pipeline --------
        def emit_front(b, st, bk):
            dept, bertST, maddT = st["dept"], st["bertST"], st["maddT"]

            # ---- A2 = bertS @ (G0 Wz)^T (two chunks through a 1-bank ring) --
            a2b = spool.tile([128, H], BF16, tag="a2b")
            for ns in (slice(0, 512), slice(512, H)):
                p_a = p_apool.tile([128, 512], F32, tag="p_a")
                w = ns.stop - ns.start
                for kc in range(KC):
                    nc.tensor.matmul(p_a[:, 0:w], bertST[:, kc, :],
                                     g0wT[:, kc, ns],
                                     start=(kc == 0), stop=(kc == KC - 1))
                nc.scalar.copy(a2b[:, ns], p_a[:, 0:w])
            if b == 0:
                dump("d_a2b", a2b[:])

            # p_t [128,1024] = 2 banks: [0:768] temp accum; [768:896] s_i row
            # then (WAR) sj+madd bcast; [896:1024] s_j row then (WAR) si col
            # at 1023 and upd col at 1022.
            p_t = p_big.tile([128, 1024], F32, tag="p_big")

            # ---- s_i / s_j rows (m=1 each; DVE can't read partition 1) ----
            for kc in range(KC):
                nc.tensor.matmul(p_t[0:1, 768:896], u2T[:, kc, 0:1],
                                 bertST[:, kc, :],
                                 start=(kc == 0), stop=(kc == KC - 1))
            for kc in range(KC):
                nc.tensor.matmul(p_t[0:1, 896:1024], u2T[:, kc, 1:2],
                                 bertST[:, kc, :],
                                 start=(kc == 0), stop=(kc == KC - 1))
            si_row = spool.tile([1, 128], F32, tag="si_row")
            nc.scalar.copy(si_row[:], p_t[0:1, 768:896])
            sjb = spool.tile([1, 128], F32, tag="sjb")
            nc.vector.tensor_scalar(sjb[:], p_t[0:1, 896:1024], bar[0:1, 0:1],
                                    None, op0=OP.add)
            # sj row bcast + additive mask (WAR over s_i region), si col
            nc.tensor.matmul(p_t[:, 768:896], maddT[:], id_bf[:],
                             start=True, stop=False)
            nc.tensor.matmul(p_t[:, 768:896], ones_f[:], sjb[:],
                             start=False, stop=True)
            nc.tensor.matmul(p_t[:, 1023:1024], si_row[:], ones_f[0:1, 0:1],
                             start=True, stop=True)
            if b == 0:
                dump("d_si", si_row[:])
                dump("d_sjb", sjb[:])

            # ---- temp1 = bertS @ (WhZ Wz - I)^T + brow ----
            for ns in (slice(0, 512), slice(512, H)):
                for kc in range(KC):
                    nc.tensor.matmul(p_t[:, ns], bertST[:, kc, :],
                                     m1T[:, kc, ns],
                                     start=(kc == 0), stop=False)
                nc.tensor.matmul(p_t[:, ns], ones_b[:], brow[0:1, ns],
                                 start=False, stop=False)

            # ---- s_e: 6-level TT tree over the middle e axis.  Every level
            # keeps FULL-j (256B) innermost runs -> DVE 2x packed mode; the
            # DVE/GpSimd split is by e-rows, never by j. ----
            seA = tpool.tile([128, 32, 128], BF16, tag="seA")
            seB = tpool.tile([128, 16, 128], BF16, tag="seB")
            nc.vector.tensor_tensor(seA[:, 0:L1G, :], dept[:, 0:L1G, :],
                                    dept[:, 32:32 + L1G, :], op=OP.add)
            nc.gpsimd.tensor_tensor(seA[:, L1G:32, :], dept[:, L1G:32, :],
                                    dept[:, 32 + L1G:64, :], op=OP.add)
            nc.vector.tensor_tensor(seB[:, 0:L2G, :], seA[:, 0:L2G, :],
                                    seA[:, 16:16 + L2G, :], op=OP.add)
            nc.gpsimd.tensor_tensor(seB[:, L2G:16, :], seA[:, L2G:16, :],
                                    seA[:, 16 + L2G:32, :], op=OP.add)
            nc.vector.tensor_tensor(seA[:, 0:8, :], seB[:, 0:8, :],
                                    seB[:, 8:16, :], op=OP.add)
            nc.vector.tensor_tensor(seB[:, 0:4, :], seA[:, 0:4, :],
                                    seA[:, 4:8, :], op=OP.add)
            nc.vector.tensor_tensor(seA[:, 0:2, :], seB[:, 0:2, :],
                                    seB[:, 2:4, :], op=OP.add)
            sef = spool.tile([128, 128], BF16, tag="sef")
            nc.vector.tensor_tensor(sef[:], seA[:, 0, :], seA[:, 1, :],
                                    op=OP.add)
            if b == 0:
                dump("d_se", sef[:])

            # ---- back-PE of the previous batch HERE: dT/g1/upd execute
            # during this batch's DVE tree, and sit AHEAD of this batch's
            # attnT transpose in the PE queue (which waits on attnb). ----
            if bk is not None:
                pb_, stp = bk
                nc.tensor.transpose(stp["ptb"][0:E, 128:256], stp["dvb"][:],
                                    id_bf[:])
                dT = spool.tile([E, 128], BF16, tag="dT")
                nc.scalar.copy(dT[:], stp["ptb"][0:E, 128:256])
                for ns in (slice(0, 512), slice(512, H)):
                    nc.tensor.matmul(stp["p_t"][:, ns], dT[:], g1[:, ns],
                                     start=False, stop=True)
                nc.tensor.matmul(stp["p_t"][:, 1022:1023],
                                 vrow4[0:1, pb_, :], ones_b[0:1, 0:1],
                                 start=True, stop=True)

            # ---- score = lrelu(se + si + sj + madd); softmax (no rowmax) ----
            sadd = spool.tile([128, L], F32, tag="sadd")
            nc.vector.scalar_tensor_tensor(
                sadd[:], sef[:], p_t[:, 1023:1024], p_t[:, 768:896],
                op0=OP.add, op1=OP.add)
            score = spool.tile([128, L], F32, tag="score")
            nc.vector.scalar_tensor_tensor(
                score[:], sadd[:], 0.01, sadd[:], op0=OP.mult, op1=OP.max)
            if b == 0:
                dump("d_sadd", score[:])
            ex = spool.tile([128, L], BF16, tag="ex")
            sumex = spool.tile([128, 1], F32, tag="sumex")
            nc.scalar.activation(ex[:], score[:], AF.Exp, bias=0.0,
                                 scale=1.0, accum_out=sumex[:])

            # ---- blend of the previous batch fills the DVE's exp bubble ----
            if bk is not None:
                pb_, stp = bk
                outt = opool.tile([128, H], F32, tag="outt")
                nc.vector.scalar_tensor_tensor(
                    outt[:], stp["p_t"][:, 0:H], stp["p_t"][:, 1022:1023],
                    stp["bertS"][:], op0=OP.mult, op1=OP.add)
                nc.sync.dma_start(out[pb_, 1:128, :], outt[0:127, :])
                nc.sync.dma_start(out[pb_, 0:1, :], outt[127:128, :])

            sume = spool.tile([128, 1], F32, tag="sume")
            nc.vector.tensor_scalar(sume[:], sumex[:], 1e-30, None,
                                    op0=OP.add)
            rec = spool.tile([128, 1], F32, tag="rec")
            nc.vector.reciprocal(rec[:], sume[:])
            attnb = spool.tile([128, L], BF16, tag="attnb")
            nc.vector.tensor_scalar(attnb[:], ex[:], rec[0:128, 0:1], None,
                                    op0=OP.mult)
            if b == 0:
                dump("d_attn", attnb[:])

            # attn^T via PE (bf16 PSUM ring shared with the dT transpose)
            ptb = p_tb.tile([128, 256], BF16, tag="p_tb")
            nc.tensor.transpose(ptb[:, 0:128], attnb[:], id_bf[:])
            attnT = spool.tile([128, 128], BF16, tag="attnT")
            # GpSimd's mult share reads an ACT-materialized broadcast (Pool's
            # stride-0 reads measured ~3x slower than contiguous).
            attnW = tpool.tile([128, E - EMV, 128], BF16, tag="attnW")
            nc.scalar.copy(
                attnW[:],
                attnb[:].unsqueeze(1).broadcast_to([128, E - EMV, 128]))
            nc.scalar.copy(attnT[:], ptb[:, 0:128])

            # ---- D-mult: tmp[i,e,j] = dep'[i,e,j] * attn[i,j] (DVE 2x) ----
            tmpD = tpool.tile([128, E, 128], BF16, tag="tmpD")
            nc.vector.tensor_tensor(
                tmpD[:, 0:EMV, :], dept[:, 0:EMV, :],
                attnb[:].unsqueeze(1).broadcast_to([128, EMV, 128]),
                op=OP.mult)
            nc.gpsimd.tensor_tensor(
                tmpD[:, EMV:E, :], dept[:, EMV:E, :], attnW[:], op=OP.mult)

            # ---- attn @ A2 into p_t ----
            for ns in (slice(0, 512), slice(512, H)):
                nc.tensor.matmul(p_t[:, ns], attnT[:], a2b[:, ns],
                                 start=False, stop=False)

            # ---- D-reduce over innermost j: DVE single reduce for the head,
            # GpSimd TT tree for the e-tail (Pool has no free-axis reduce) --
            dvb = spool.tile([128, E], BF16, tag="dvb")
            nc.vector.tensor_reduce(dvb[:, 0:EDR], tmpD[:, 0:EDR, :],
                                    axis=AX.X, op=OP.add)
            tDs = tpool.tile([128, E - EDR, 64], BF16, tag="tDs")
            nc.gpsimd.tensor_tensor(tDs[:, :, :], tmpD[:, EDR:E, 0:64],
                                    tmpD[:, EDR:E, 64:128], op=OP.add)
            nc.gpsimd.tensor_tensor(tmpD[:, EDR:E, 0:32], tDs[:, :, 0:32],
                                    tDs[:, :, 32:64], op=OP.add)
            nc.gpsimd.tensor_tensor(tDs[:, :, 0:16], tmpD[:, EDR:E, 0:16],
                                    tmpD[:, EDR:E, 16:32], op=OP.add)
            nc.gpsimd.tensor_tensor(tmpD[:, EDR:E, 0:8], tDs[:, :, 0:8],
                                    tDs[:, :, 8:16], op=OP.add)
            nc.gpsimd.tensor_tensor(tDs[:, :, 0:4], tmpD[:, EDR:E, 0:4],
                                    tmpD[:, EDR:E, 4:8], op=OP.add)
            nc.gpsimd.tensor_tensor(tmpD[:, EDR:E, 0:2], tDs[:, :, 0:2],
                                    tDs[:, :, 2:4], op=OP.add)
            nc.gpsimd.tensor_tensor(dvb[:, EDR:E], tmpD[:, EDR:E, 0:1],
                                    tmpD[:, EDR:E, 1:2], op=OP.add)
            if b == 0:
                dump("d_dvec", dvb[:])

            st.update(ptb=ptb, p_t=p_t, dvb=dvb)
            return st

        def emit_tail(b, stp):
            # final batch: no next front to interleave with
            nc.tensor.transpose(stp["ptb"][0:E, 128:256], stp["dvb"][:],
                                id_bf[:])
            dT = spool.tile([E, 128], BF16, tag="dT")
            nc.scalar.copy(dT[:], stp["ptb"][0:E, 128:256])
            for ns in (slice(0, 512), slice(512, H)):
                nc.tensor.matmul(stp["p_t"][:, ns], dT[:], g1[:, ns],
                                 start=False, stop=True)
            nc.tensor.matmul(stp["p_t"][:, 1022:1023], vrow4[0:1, b, :],
                             ones_b[0:1, 0:1], start=True, stop=True)
            outt = opool.tile([128, H], F32, tag="outt")
            nc.vector.scalar_tensor_tensor(
                outt[:], stp["p_t"][:, 0:H], stp["p_t"][:, 1022:1023],
                stp["bertS"][:], op0=OP.mult, op1=OP.add)
            nc.sync.dma_start(out[b, 1:128, :], outt[0:127, :])
            nc.sync.dma_start(out[b, 0:1, :], outt[127:128, :])

        sts = {0: st0}
        for b in range(PB):
            bk = (b - 1, sts.pop(b - 1)) if b >= 1 else None
            sts[b] = emit_front(b, sts[b], bk)
            if b + 1 < PB:
                sts[b + 1] = prefetch(b + 1)
        emit_tail(PB - 1, sts.pop(PB - 1))


def _get_nc():
    if "nc" not in _CACHED:
        _CACHED["nc"] = _build(debug=bool(_CACHED.get("debug")))
    return _CACHED["nc"]


def _chunkT(w):
    """W [rows, K] -> W^T chunk-major [128, K//128, rows] (lhsT layout)."""
    rows, k = w.shape
    return np.ascontiguousarray(
        w.T.reshape(k // 128, 128, rows).transpose(1, 0, 2))


def _prep_in_maps(bert_hidden_states, dep_type_adj, deprel_adj,
                  asp_start, asp_end, Wz, bz, wa, ba, Wf, Wh):
    bf = ml_dtypes.bfloat16
    bert = np.ascontiguousarray(np.asarray(bert_hidden_states, np.float32))
    wa_f = np.asarray(wa, np.float32)
    wa_i, wa_j, wae_f = wa_f[:H], wa_f[H:2 * H], wa_f[2 * H:]
    wae_safe = np.where(wae_f == 0.0, 1.0, wae_f)
    # dep': wa_e folded in, transposed to [b, i, e, j]
    depW = np.asarray(dep_type_adj, np.float32) * wae_f[None, None, None, :]
    dept = np.ascontiguousarray(depW.transpose(0, 1, 3, 2)).astype(bf)
    adjn = np.asarray(deprel_adj) > 0
    madd = np.where(adjn, np.float32(0.0), np.float32(MASK_NEG))
    maddT = np.ascontiguousarray(madd.transpose(0, 2, 1)).astype(bf)
    # bertS^T chunk-major per batch: rows shifted by one (the z-roll)
    bs = np.ascontiguousarray(np.roll(bert, -1, axis=1))
    bertsT = np.ascontiguousarray(
        bs.transpose(0, 2, 1).reshape(B, KC, 128, L).transpose(0, 2, 1, 3)
    ).astype(bf)
    pos = np.arange(L, dtype=np.float32)
    s_ = np.asarray(asp_start).astype(np.float32)[:, None]
    e_ = np.asarray(asp_end).astype(np.float32)[:, None]
    vrow_full = (((pos[None, :] >= s_) & (pos[None, :] <= e_))
                 & adjn.any(-1)).astype(bf)

    Wz = np.asarray(Wz, np.float32)
    bz_f = np.asarray(bz, np.float32)
    ba_f = np.float32(np.asarray(ba, np.float32))
    Wf = np.asarray(Wf, np.float32)
    Wh = np.asarray(Wh, np.float32)
    WfZ, WfE = Wf[:, :H], Wf[:, H:]
    WhN, WhZ = Wh[:, :H], Wh[:, H:]
    G0 = WhN @ WfZ
    g0wT = _chunkT(G0 @ Wz).astype(bf)
    m1T = _chunkT(WhZ @ Wz - np.eye(H, dtype=np.float32)).astype(bf)
    g1 = np.ascontiguousarray(
        (WhN @ WfE).T / wae_safe[:, None]).astype(bf)
    u2 = np.stack([Wz.T @ wa_i, Wz.T @ wa_j], axis=0)  # [2, H]
    u2T = _chunkT(u2).astype(bf)
    brow = (WhZ @ bz_f + G0 @ bz_f)[None, :].astype(bf)
    bab = np.float32(ba_f + wa_i @ bz_f + wa_j @ bz_f).reshape(1, 1)

    in_maps = []
    for c in range(NCORES):
        s = slice(c * PB, (c + 1) * PB)
        in_maps.append(dict(
            berts=bs[s], bertsT=np.ascontiguousarray(bertsT[s]),
            dept=dept[s], maddT=maddT[s],
            vrow=np.ascontiguousarray(vrow_full[s][None, :, :]),
            g0wT=g0wT, m1T=m1T, g1=g1, u2T=u2T,
            browt=brow, bat=bab,
        ))
    return in_maps


def kernel(bert_hidden_states, dep_type_adj, deprel_adj, asp_start, asp_end,
           Wz, bz, wa, ba, Wf, Wh):
    from concourse.bass_utils import run_bass_kernel_spmd

    in_maps = _prep_in_maps(bert_hidden_states, dep_type_adj, deprel_adj,
                            asp_start, asp_end, Wz, bz, wa, ba, Wf, Wh)
    nc = _get_nc()
    res = run_bass_kernel_spmd(nc, in_maps, core_ids=list(range(NCORES)),
                               trace=bool(_CACHED.get("trace")),
                               tmpdir=_CACHED.get("trace_tmpdir"))
    _CACHED["last_results"] = res
    outs = [res.results[c]["out"] for c in range(NCORES)]
    return np.concatenate(outs, axis=0).astype(np.float32)
